# revision 39
# baseline (speedup 1.0000x reference)
"""Causal self-attention Trainium2 Bass kernel, data-parallel over 8 NeuronCores.

Problem (hardcoded): x [8, 2048, 1024] fp32; w_attn [1024, 3072]; b_attn [3072];
w_proj [1024, 1024]; b_proj [1024]. H=16 heads, D=64.

Sharding: batch (8) -> one sample per core. Weights replicated. All matmul
operands are bf16 (inputs converted host-side; fp32 PSUM accumulation), which
keeps every matmul at 1 cycle/row on the PE and halves SBUF/DMA footprints.

Host-side input marshalling per core: x is pre-transposed to xT [C, T] and
w_attn's q|k halves are pre-packed so each 128-channel chunk loads as one
contiguous [128, C] lhsT tile.

Per-core dataflow (fused single pipeline, no DRAM staging):
  - qT/kT [128ch, T] = w_chunk^T @ xT, kept in SBUF (pool of 8 chunks,
    produced just in time and freed after the pair's attention)
  - v [tk, h, 64+1] in SBUF; column 64 is 1.0 (rides the PV matmul to
    produce softmax denominators)
  - per pair of heads (even/odd share a 128-channel chunk), per 512-wide
    q-block, per 128-wide k-chunk c:
      S^T merged tile [128tk, 1024]: even head at cols [n0:512] (bank 0),
      odd shifted to [512:1024-n0] (bank 1) so one Exp instruction (scale
      folded) covers both heads with no garbage zone -> ex bf16;
      causal mask = 0/1 multiply on the two diagonal 128-blocks only
      PV: ex 128-col slices are the STATIONARY operand, v [128,65] the
      moving one -> y [tq, 65] accumulated over c in PSUM (free dim 65
      instead of 512: halves PV cycles vs the yT layout); only the very
      first matmul per y bank sets start=True (start clears has_written
      for the WHOLE bank; per-element bits handle later regions)
  - normalize: per-partition reciprocal of the denominator column +
    tensor_scalar multiply (no PE broadcast needed in [tq, d] layout)
  - y -> yT via PE transpose (f32, 128x128) feeding the output projection
  - out [T, C] = y @ w_proj via lhsT = yT chunks, split into two
    contraction halves: half 0 (pairs 0-3) runs during pairs 4-6 with its
    partial sums staged in SBUF (bf16), half 1 adds them back at the end

All non-attention matmul work (qkv, v, both proj halves) is emitted through
generators interleaved into the attention c-loops (debt-paced, with
deadlock-avoiding gates) so the in-order PE queue never drains while the
ACT engine runs Exp; PV is software-pipelined one k-chunk behind S/Exp.
Cost-model makespan ~502us vs 648us for the f32r phase-sequential baseline.
"""

import numpy as np
from contextlib import ExitStack

import ml_dtypes

import concourse.bacc as bacc
import concourse.tile as tile
from concourse import mybir
from concourse.bass_utils import run_bass_kernel_spmd

F32 = mybir.dt.float32
BF16 = mybir.dt.bfloat16
BF = ml_dtypes.bfloat16
P = 128

# cost-model cycle estimates (us) used only for emission pacing
_MM_US = 0.0004167      # PE cycle @2.4GHz
_ACT_US = 0.000833      # ACT cycle @1.2GHz


def build_program(T=2048, C=1024, H=16, n_cores=8,
                  with_bias_attn=False, with_bias_proj=False):
    D = C // H            # 64
    assert D == 64 and H % 2 == 0
    CIN = C // P          # 8 contraction chunks
    PAIRS = C // P        # 8 head pairs
    TKC = T // P          # 16 k-chunks
    QB = 512
    NQB = T // QB         # 4
    SUB = QB // P         # 4
    scale = 1.0 / float(np.sqrt(D))

    nc = bacc.Bacc("TRN2", target_bir_lowering=False, debug=False,
                   num_devices=n_cores)

    xT_in = nc.dram_tensor("xT", [C, T], BF16, kind="ExternalInput")
    wqk_in = nc.dram_tensor("wqk", [2 * C, C], BF16, kind="ExternalInput")
    wv_in = nc.dram_tensor("wv", [C, C], BF16, kind="ExternalInput")
    wp_in = nc.dram_tensor("wp", [C, C], BF16, kind="ExternalInput")
    ident_in = nc.dram_tensor("ident", [P, P], F32, kind="ExternalInput")
    mask_in = nc.dram_tensor("mask", [P, P], BF16, kind="ExternalInput")
    if with_bias_attn:
        b_attn = nc.dram_tensor("b_attn", [1, 3 * C], BF16,
                                kind="ExternalInput")
    if with_bias_proj:
        b_proj = nc.dram_tensor("b_proj", [1, C], BF16, kind="ExternalInput")
    out_d = nc.dram_tensor("out", [T, C], F32, kind="ExternalOutput")

    with tile.TileContext(nc) as tc, ExitStack() as ctx:
        ctx.enter_context(nc.allow_low_precision(reason="bf16 pipeline"))
        pool_c = ctx.enter_context(tc.tile_pool(name="const", bufs=1))
        ident_t = pool_c.tile([P, P], F32, tag="ident")
        mask_t = pool_c.tile([P, P], BF16, tag="mask")
        nc.sync.dma_start(ident_t[:], ident_in[:])
        nc.sync.dma_start(mask_t[:], mask_in[:])
        if with_bias_attn:
            ba_t = pool_c.tile([1, 3 * C], BF16, tag="ba")
            nc.sync.dma_start(ba_t[:], b_attn[:])
        if with_bias_proj:
            bp_t = pool_c.tile([1, C], BF16, tag="bp")
            nc.sync.dma_start(bp_t[:], b_proj[:])
        if with_bias_attn or with_bias_proj:
            ones_row = pool_c.tile([1, 512], BF16, tag="ones_row")
            nc.gpsimd.memset(ones_row[:], 1.0)

        # resident tensors
        pool_xT = ctx.enter_context(tc.tile_pool(name="xT", bufs=1))
        xT_sb = [pool_xT.tile([P, T], BF16, tag=f"xT{j}", name=f"xT{j}")
                 for j in range(CIN)]
        pool_v = ctx.enter_context(tc.tile_pool(name="vres", bufs=1))
        v_t = [pool_v.tile([P, H, D + 1], BF16, tag=f"v{i}", name=f"v{i}")
               for i in range(TKC)]
        pool_y = ctx.enter_context(tc.tile_pool(name="yres", bufs=1))
        yT_sb = [pool_y.tile([P, T], BF16, tag=f"y{j}", name=f"y{j}")
                 for j in range(CIN)]

        # working pools
        pool_qk = ctx.enter_context(tc.tile_pool(name="qkpool", bufs=8))
        pool_wqk = ctx.enter_context(tc.tile_pool(name="wqk", bufs=2))
        pool_wv = ctx.enter_context(tc.tile_pool(name="wv", bufs=1))
        pool_ex = ctx.enter_context(tc.tile_pool(name="expool", bufs=4))
        pool_yn = ctx.enter_context(tc.tile_pool(name="ynpool", bufs=6))
        pool_rc = ctx.enter_context(tc.tile_pool(name="rcpool", bufs=8))
        pool_ost = ctx.enter_context(tc.tile_pool(name="ostpool", bufs=4))
        pool_part = ctx.enter_context(tc.tile_pool(name="partpool", bufs=32))

        # PSUM: sT 2x2 banks + y 2x1 + mm 2x1 = 8 banks exactly
        psum_s = ctx.enter_context(
            tc.tile_pool(name="ps_s", bufs=2, space="PSUM"))
        psum_y = ctx.enter_context(
            tc.tile_pool(name="ps_y", bufs=2, space="PSUM"))
        psum_mm = ctx.enter_context(
            tc.tile_pool(name="ps_mm", bufs=2, space="PSUM"))

        for i in range(TKC):
            nc.gpsimd.memset(v_t[i][:, :, D:D + 1], 1.0)
        for j in range(CIN):
            nc.sync.dma_start(xT_sb[j][:], xT_in[j * P:(j + 1) * P, :])

        qkT = {}        # pr -> (qT tile, kT tile)
        parts = []      # staged proj half-0 partial sums (FIFO)
        state = {"v_prog": {0: -1, 1: -1}, "qk_prog": {}, "tr7": 0,
                 "tr": {}, "cur_pair": -1}

        # ---------------- feeder generators ----------------
        def gen_qkv(pr):
            # qT_pr reuses qT_{pr-4}'s SBUF slot (bufs=8), whose last reader
            # is attention(pr-4)'s S matmul: emitting this pair's copies
            # before attention(pr-2) starts can cycle the in-order DVE queue
            # against PE (observed deadlock) — gate on attention progress.
            while state["cur_pair"] < pr - 2:
                yield None
            qt = pool_qk.tile([P, T], BF16, tag="qk", name=f"qT{pr}")
            kt = pool_qk.tile([P, T], BF16, tag="qk", name=f"kT{pr}")
            qkT[pr] = (qt, kt)
            state["qk_prog"][pr] = -1
            wms = {}
            for m in (pr, PAIRS + pr):
                wm = pool_wqk.tile([P, CIN, P], BF16, tag="wqk", name="wm")
                nc.sync.dma_start(
                    wm[:],
                    wqk_in[m * P:(m + 1) * P, :].rearrange(
                        "p (j n) -> p j n", n=P))
                wms[m] = wm
            # q/k interleaved per 512-wide t-slice so attention(pr) qb j can
            # start as soon as slices <= j exist
            for tt in range(T // 512):
                for m, dst in ((pr, qt), (PAIRS + pr, kt)):
                    ps = psum_mm.tile([P, 512], F32, tag="mm", name="ps_qk")
                    for j in range(CIN):
                        nc.tensor.matmul(
                            ps[:], wms[m][:, j, :],
                            xT_sb[j][:, tt * 512:(tt + 1) * 512],
                            start=(j == 0),
                            stop=(j == CIN - 1 and not with_bias_attn))
                    if with_bias_attn:
                        col0 = m * P if m < PAIRS else C + (m - PAIRS) * P
                        nc.tensor.matmul(
                            ps[:], ba_t[0:1, col0:col0 + P],
                            ones_row[0:1, :], start=False, stop=True)
                    nc.vector.tensor_copy(
                        dst[:, tt * 512:(tt + 1) * 512], ps[:])
                    yield 1.8
                state["qk_prog"][pr] = tt

        def gen_v(slab):
            # slab 1 feeds pairs 4-7 only: hold its emission back so it can
            # fill the late pairs' exp-latency gaps instead of the early ones
            # (pair 4's per-qb barriers force what they need just in time)
            while slab == 1 and state["cur_pair"] < 3:
                yield None
            g = slab * 512
            wv_t = []
            for j in range(CIN):
                wv = pool_wv.tile([P, 512], BF16, tag=f"wv{j}", name=f"wv{j}")
                nc.sync.dma_start(
                    wv[:], wv_in[j * P:(j + 1) * P, g:g + 512])
                wv_t.append(wv)
            for i in range(TKC):
                ps = psum_mm.tile([P, 512], F32, tag="mm", name="ps_v")
                for j in range(CIN):
                    nc.tensor.matmul(
                        ps[:], xT_sb[j][:, i * P:(i + 1) * P], wv_t[j][:],
                        start=(j == 0),
                        stop=(j == CIN - 1 and not with_bias_attn))
                if with_bias_attn:
                    nc.tensor.matmul(
                        ps[:], ones_row[0:1, 0:P],
                        ba_t[0:1, 2 * C + g:2 * C + g + 512],
                        start=False, stop=True)
                nc.vector.tensor_copy(
                    v_t[i][:, g // D:(g + 512) // D, 0:D],
                    ps[:].rearrange("p (h d) -> p h d", d=D))
                state["v_prog"][slab] = i
                yield 1.8

        def gen_proj_half(h):
            # contraction split: half 0 (yT chunks 0-3) only needs pairs 0-3
            # and becomes PE fill for the otherwise-starved pairs 4-6; its
            # partial sums stage in SBUF (bf16) and half 1 adds them back.
            # Gate: pair h-half's transposes are all emitted once cur_pair
            # moves past the half's last pair (stage2 drains at pair end).
            while state["cur_pair"] < (3 if h == 0 else PAIRS - 1):
                yield None
            js = list(range(4 * h, 4 * h + 4))
            for g in (0, 512):
                wp_t = {}
                for j in js:
                    wp = pool_wv.tile([P, 512], BF16, tag=f"wp{j}",
                                      name=f"wp{j}")
                    nc.sync.dma_start(
                        wp[:], wp_in[j * P:(j + 1) * P, g:g + 512])
                    wp_t[j] = wp
                for i in range(TKC):
                    last_pr = 4 * h + 3
                    while (state["cur_pair"] <= last_pr and
                           state["tr"].get(last_pr, 0) < i + 1):
                        yield None
                    ps = psum_mm.tile([P, 512], F32, tag="mm", name="ps_o")
                    for j in js:
                        nc.tensor.matmul(
                            ps[:], yT_sb[j][:, i * P:(i + 1) * P], wp_t[j][:],
                            start=(j == js[0]),
                            stop=(j == js[-1] and not
                                  (h == 1 and with_bias_proj)))
                    if h == 0:
                        part = pool_part.tile([P, 512], BF16, tag="part",
                                              name="part")
                        nc.vector.tensor_copy(part[:], ps[:])
                        parts.append(part)
                    else:
                        if with_bias_proj:
                            nc.tensor.matmul(
                                ps[:], ones_row[0:1, 0:P],
                                bp_t[0:1, g:g + 512],
                                start=False, stop=True)
                        ost = pool_ost.tile([P, 512], F32, tag="ost",
                                            name="ost")
                        nc.vector.tensor_add(ost[:], ps[:],
                                             parts.pop(0)[:])
                        nc.sync.dma_start(
                            out_d[i * P:(i + 1) * P, g:g + 512], ost[:])
                    yield 1.0

        class Feeder:
            def __init__(self):
                self.gens = []

            def push(self, g):
                self.gens.append(g)

            def pull_one(self):
                """Advance one unit from the first non-blocked generator
                (blocked heads are skipped, order otherwise preserved).
                Returns cost (us), 0.0 if all blocked, None if exhausted."""
                idx = 0
                while idx < len(self.gens):
                    try:
                        cost = next(self.gens[idx])
                    except StopIteration:
                        self.gens.pop(idx)
                        continue
                    if cost is None:
                        idx += 1        # gated — try the next generator
                        continue
                    return cost
                return None if not self.gens else 0.0

            def drain(self):
                while self.pull_one() is not None:
                    pass

        feeder = Feeder()

        # ---------------- attention ----------------
        def attention(pr):
            state["cur_pair"] = pr
            qT, kT = qkT[pr]
            slab = pr // 4
            pending1 = []   # stage1: normalize (DVE)
            pending2 = []   # stage2: transpose + yT copy (PE+DVE)

            def emit_stage1(y_e, y_o, s, i):
                rc_e = pool_rc.tile([P, 1], F32, tag="rc", name="rc_e")
                rc_o = pool_rc.tile([P, 1], F32, tag="rc", name="rc_o")
                yn = pool_yn.tile([P, P], F32, tag="yn", name="yn")
                with nc.allow_low_precision(reason="softmax normalize"):
                    nc.vector.reciprocal(rc_e[:], y_e[:, 65 * s + D:65 * s + D + 1])
                    nc.vector.reciprocal(rc_o[:], y_o[:, 65 * s + D:65 * s + D + 1])
                    nc.vector.tensor_scalar(
                        yn[:, 0:D], y_e[:, 65 * s:65 * s + D], rc_e[:, 0:1],
                        None, op0=mybir.AluOpType.mult)
                    nc.vector.tensor_scalar(
                        yn[:, D:2 * D], y_o[:, 65 * s:65 * s + D], rc_o[:, 0:1],
                        None, op0=mybir.AluOpType.mult)
                pending2.append((yn, i))

            def emit_stage2(yn, i):
                # PE transpose (DMA-engine transposes would head-block either
                # hwdge queue on the yn dependency: SP deadlocks vs wv/out
                # DMAs, ACT stalls exp dispatch)
                tr = psum_mm.tile([P, 512], F32, tag="mm", name="tr")
                nc.tensor.transpose(tr[:, 0:P], yn[:], ident_t[:])
                nc.vector.tensor_copy(yT_sb[pr][:, i * P:(i + 1) * P],
                                      tr[:, 0:P])
                state["tr"][pr] = i + 1
                if pr == PAIRS - 1:
                    state["tr7"] = i + 1

            debt = [0.0]

            def fill(extra=0.0):
                # in the last pair every pulled unit shortens the otherwise
                # ACT-idle projection tail: pull as hard as supply allows
                last = pr == PAIRS - 1
                debt[0] += extra + (0.25 if last else 0.0)
                pulls = 0
                while debt[0] > 0 and pulls < (4 if last else 2):
                    cost = feeder.pull_one()
                    if not cost:
                        break
                    debt[0] -= cost
                    pulls += 1
                debt[0] = min(debt[0], 8.0 if last else 4.0)

            for qb in range(NQB):
                q0 = qb * QB
                # this qb's q/k slices and v chunks must already be emitted
                spins = 0
                while (state["qk_prog"][pr] < qb or
                       state["v_prog"][slab] < min(4 * qb + 3, TKC - 1)):
                    c_ = feeder.pull_one()
                    spins += 1
                    if c_ is None or spins > 100000:
                        raise RuntimeError("feeder stuck before qk/v ready")
                y_e = psum_y.tile([P, 512], F32, tag="y", name="y_e")
                y_o = psum_y.tile([P, 512], F32, tag="y", name="y_o")

                def emit_pv(ex, n0, c, y_e=y_e, y_o=y_o, qb=qb):
                    s_min = max(0, c - 4 * qb)
                    for s in range(s_min, SUB):
                        # start=True clears has_written for the WHOLE bank, so
                        # only the tile's very first matmul may set it; later
                        # regions first-write via the cleared per-element bits
                        st_ = (c == 0 and s == 0)
                        sp_ = (c == 4 * qb + s)
                        nc.tensor.matmul(
                            y_e[:, 65 * s:65 * s + 65],
                            ex[:, s * P:(s + 1) * P],
                            v_t[c][:, 2 * pr, :],
                            start=st_, stop=sp_, skip_group_check=True)
                        nc.tensor.matmul(
                            y_o[:, 65 * s:65 * s + 65],
                            ex[:, QB + s * P - n0:QB + (s + 1) * P - n0],
                            v_t[c][:, 2 * pr + 1, :],
                            start=st_, stop=sp_, skip_group_check=True)
                        if sp_:
                            pending1.append((y_e, y_o, s, 4 * qb + s))

                pv_queue = []
                for c in range(4 * qb + 4):
                    n0 = max(0, c * P - q0)
                    sT = psum_s.tile([P, 2 * QB], F32, tag="sT", name="sT")
                    nc.tensor.matmul(
                        sT[:, n0:QB],
                        kT[0:D, c * P:(c + 1) * P],
                        qT[0:D, q0 + n0:q0 + QB],
                        start=True, stop=True, tile_position=(0, 0))
                    nc.tensor.matmul(
                        sT[:, QB:2 * QB - n0],
                        kT[D:2 * D, c * P:(c + 1) * P],
                        qT[D:2 * D, q0 + n0:q0 + QB],
                        start=True, stop=True, tile_position=(D, 0))
                    if pending2:
                        emit_stage2(*pending2.pop(0))
                    if pending1:
                        emit_stage1(*pending1.pop(0))
                    ex = pool_ex.tile([P, 2 * QB], BF16, tag="ex", name="ex")
                    nc.scalar.activation(ex[:, n0:2 * QB - n0],
                                         sT[:, n0:2 * QB - n0],
                                         mybir.ActivationFunctionType.Exp,
                                         scale=scale)
                    if c * P >= q0:   # diagonal 128-block: causal 0/1 mask
                        nc.vector.tensor_mul(ex[:, n0:n0 + P],
                                             ex[:, n0:n0 + P], mask_t[:])
                        nc.vector.tensor_mul(ex[:, QB:QB + P],
                                             ex[:, QB:QB + P], mask_t[:])
                    # debt-paced feeder fill: keep PE busy while ACT exps
                    s_min = max(0, c - 4 * qb)
                    act_c = (2 * QB - 2 * n0) * _ACT_US + 0.30
                    pe_c = (2 * (QB - n0) +
                            2 * (SUB - s_min) * 65) * _MM_US + 0.06
                    fill(act_c - pe_c)
                    # software pipeline (depth 2): PV lags the S/exp front by
                    # two chunks so the PE never couples to exp completion
                    pv_queue.append((ex, n0, c))
                    if len(pv_queue) > 1:
                        emit_pv(*pv_queue.pop(0))
                for it in pv_queue:
                    emit_pv(*it)
                # the y_e/y_o PSUM slots are recycled by the next qb's
                # allocation: every pending normalize reading them must be
                # emitted before that (stage2 may stay pending)
                while pending1:
                    emit_stage1(*pending1.pop(0))
                    feeder.pull_one()
            # drain transposes, interleaving feeder units
            while pending2:
                emit_stage2(*pending2.pop(0))
                feeder.pull_one()

        # ---------------- program ----------------
        # lead-in: pair 0's first q/k slice + first 4 v chunks directly
        g0 = gen_qkv(0)
        while state["qk_prog"].get(0, -1) < 0:
            next(g0)
        gv0 = gen_v(0)
        while state["v_prog"][0] < SUB - 1:
            next(gv0)
        feeder.push(g0)
        feeder.push(gv0)
        feeder.push(gen_qkv(1))
        feeder.push(gen_qkv(2))
        feeder.push(gen_qkv(3))
        feeder.push(gen_v(1))
        feeder.push(gen_qkv(4))
        feeder.push(gen_qkv(5))
        feeder.push(gen_qkv(6))
        feeder.push(gen_qkv(7))
        feeder.push(gen_proj_half(0))
        feeder.push(gen_proj_half(1))

        for pr in range(PAIRS):
            # barrier: this pair's first q/k slice must be emitted
            spins = 0
            while state["qk_prog"].get(pr, -1) < 0:
                c_ = feeder.pull_one()
                spins += 1
                if c_ is None or spins > 100000:
                    raise RuntimeError("feeder stuck before qk ready")
            attention(pr)
        feeder.drain()

    nc.compile()
    return nc


def make_const_inputs():
    ident = np.eye(P, dtype=np.float32)
    # S^T diagonal block mask: valid iff tq_local >= tk_local
    mask = np.triu(np.ones((P, P), np.float32)).astype(BF)
    return ident, mask


def make_in_maps(inputs, n_cores=8):
    """Host-side marshalling: shard x over batch, convert to bf16, transpose
    x, pack w_attn's q|k chunks into contiguous lhsT tiles."""
    x = np.asarray(inputs["x"], dtype=np.float32)
    w_attn = np.asarray(inputs["w_attn"], dtype=np.float32)
    w_proj = np.asarray(inputs["w_proj"], dtype=np.float32)
    b_attn = np.asarray(inputs.get("b_attn", 0), dtype=np.float32)
    b_proj = np.asarray(inputs.get("b_proj", 0), dtype=np.float32)
    B, T, C = x.shape

    wqk = w_attn[:, :2 * C]        # [C, 2C]
    # chunk m tile [p, j*128+n] = w_attn[j*128+p, m*128+n]
    wqk_packed = np.ascontiguousarray(
        wqk.reshape(C // P, P, 2 * C // P, P)     # [j, p, m, n]
        .transpose(2, 1, 0, 3)                    # [m, p, j, n]
        .reshape(2 * C, C)).astype(BF)
    wv = np.ascontiguousarray(w_attn[:, 2 * C:]).astype(BF)
    wp = np.ascontiguousarray(w_proj).astype(BF)
    ident, mask = make_const_inputs()

    wba = bool(np.any(b_attn != 0))
    wbp = bool(np.any(b_proj != 0))
    in_maps = []
    for i in range(n_cores):
        m = {"xT": np.ascontiguousarray(x[i].T).astype(BF),
             "wqk": wqk_packed, "wv": wv, "wp": wp,
             "ident": ident, "mask": mask}
        if wba:
            m["b_attn"] = b_attn.reshape(1, -1).astype(BF)
        if wbp:
            m["b_proj"] = b_proj.reshape(1, -1).astype(BF)
        in_maps.append(m)
    return in_maps


_CACHE = {}


def _get_program(T, C, H, wba, wbp, n_cores):
    key = (T, C, H, wba, wbp, n_cores)
    if key not in _CACHE:
        _CACHE[key] = build_program(T=T, C=C, H=H, n_cores=n_cores,
                                    with_bias_attn=wba, with_bias_proj=wbp)
    return _CACHE[key]


def kernel(x, w_attn, b_attn, w_proj, b_proj):
    x = np.asarray(x, dtype=np.float32)
    B, T, C = x.shape
    H = 16
    n_cores = 8
    assert B == n_cores

    inputs = {"x": x, "w_attn": w_attn, "b_attn": b_attn,
              "w_proj": w_proj, "b_proj": b_proj}
    in_maps = make_in_maps(inputs, n_cores)
    wba = "b_attn" in in_maps[0]
    wbp = "b_proj" in in_maps[0]
    nc = _get_program(T, C, H, wba, wbp, n_cores)

    res = run_bass_kernel_spmd(nc, in_maps, list(range(n_cores)))
    return np.stack([res.results[i]["out"] for i in range(n_cores)], axis=0)


# revision 42
# speedup vs baseline: 1.0528x; 1.0528x over previous
"""Causal self-attention Trainium2 Bass kernel, data-parallel over 8 NeuronCores.

Problem (hardcoded): x [8, 2048, 1024] fp32; w_attn [1024, 3072]; b_attn [3072];
w_proj [1024, 1024]; b_proj [1024]. H=16 heads, D=64.

Sharding: batch (8) -> one sample per core. Weights replicated. All matmul
operands are bf16 (inputs converted host-side; fp32 PSUM accumulation), which
keeps every matmul at 1 cycle/row on the PE and halves SBUF/DMA footprints.

Host-side input marshalling per core: x is pre-transposed to xT [C, T] and
w_attn's q|k halves are pre-packed so each 128-channel chunk loads as one
contiguous [128, C] lhsT tile.

Per-core dataflow (fused single pipeline, no DRAM staging):
  - qT/kT [128ch, T] = w_chunk^T @ xT, kept in SBUF (pool of 8 chunks,
    produced just in time and freed after the pair's attention)
  - v [tk, h, 64+1] in SBUF; column 64 is 1.0 (rides the PV matmul to
    produce softmax denominators)
  - per pair of heads (even/odd share a 128-channel chunk), per 512-wide
    q-block, per 128-wide k-chunk c:
      S^T merged tile [128tk, 1024]: even head at cols [n0:512] (bank 0),
      odd shifted to [512:1024-n0] (bank 1) so one Exp instruction (scale
      folded) covers both heads with no garbage zone -> ex bf16;
      causal mask = 0/1 multiply on the two diagonal 128-blocks only
      PV: ex 128-col slices are the STATIONARY operand, v [128,65] the
      moving one -> y [tq, 65] accumulated over c in PSUM (free dim 65
      instead of 512: halves PV cycles vs the yT layout); only the very
      first matmul per y bank sets start=True (start clears has_written
      for the WHOLE bank; per-element bits handle later regions)
  - normalize: per-partition reciprocal of the denominator column +
    tensor_scalar multiply (no PE broadcast needed in [tq, d] layout)
  - y -> yT via PE transpose (f32, 128x128) feeding the output projection
  - out [T, C] = y @ w_proj via lhsT = yT chunks, split into two
    contraction halves: half 0 (pairs 0-3) runs during pairs 4-6 with its
    partial sums staged in SBUF (bf16), half 1 adds them back at the end

All non-attention matmul work (qkv, v, both proj halves) is emitted through
generators interleaved into the attention c-loops (debt-paced, with
deadlock-avoiding gates) so the in-order PE queue never drains while the
ACT engine runs Exp; PV is software-pipelined one k-chunk behind S/Exp.
Cost-model makespan ~502us vs 648us for the f32r phase-sequential baseline.
"""

import numpy as np
from contextlib import ExitStack

import ml_dtypes

import concourse.bacc as bacc
import concourse.tile as tile
from concourse import mybir
from concourse.bass_utils import run_bass_kernel_spmd

F32 = mybir.dt.float32
BF16 = mybir.dt.bfloat16
BF = ml_dtypes.bfloat16
P = 128

# cost-model cycle estimates (us) used only for emission pacing
_MM_US = 0.0004167      # PE cycle @2.4GHz
_ACT_US = 0.000833      # ACT cycle @1.2GHz


def build_program(T=2048, C=1024, H=16, n_cores=8,
                  with_bias_attn=False, with_bias_proj=False):
    D = C // H            # 64
    assert D == 64 and H % 2 == 0
    CIN = C // P          # 8 contraction chunks
    PAIRS = C // P        # 8 head pairs
    TKC = T // P          # 16 k-chunks
    QB = 512
    NQB = T // QB         # 4
    SUB = QB // P         # 4
    scale = 1.0 / float(np.sqrt(D))

    nc = bacc.Bacc("TRN2", target_bir_lowering=False, debug=False,
                   num_devices=n_cores)

    xT_in = nc.dram_tensor("xT", [C, T], BF16, kind="ExternalInput")
    wqk_in = nc.dram_tensor("wqk", [2 * C, C], BF16, kind="ExternalInput")
    wv_in = nc.dram_tensor("wv", [C, C], BF16, kind="ExternalInput")
    wp_in = nc.dram_tensor("wp", [C, C], BF16, kind="ExternalInput")
    ident_in = nc.dram_tensor("ident", [P, P], F32, kind="ExternalInput")
    mask_in = nc.dram_tensor("mask", [P, P], BF16, kind="ExternalInput")
    if with_bias_attn:
        b_attn = nc.dram_tensor("b_attn", [1, 3 * C], BF16,
                                kind="ExternalInput")
    if with_bias_proj:
        b_proj = nc.dram_tensor("b_proj", [1, C], BF16, kind="ExternalInput")
    out_d = nc.dram_tensor("out", [T, C], F32, kind="ExternalOutput")

    with tile.TileContext(nc) as tc, ExitStack() as ctx:
        ctx.enter_context(nc.allow_low_precision(reason="bf16 pipeline"))
        pool_c = ctx.enter_context(tc.tile_pool(name="const", bufs=1))
        ident_t = pool_c.tile([P, P], F32, tag="ident")
        mask_t = pool_c.tile([P, P], BF16, tag="mask")
        nc.sync.dma_start(ident_t[:], ident_in[:])
        nc.sync.dma_start(mask_t[:], mask_in[:])
        if with_bias_attn:
            ba_t = pool_c.tile([1, 3 * C], BF16, tag="ba")
            nc.sync.dma_start(ba_t[:], b_attn[:])
        if with_bias_proj:
            bp_t = pool_c.tile([1, C], BF16, tag="bp")
            nc.sync.dma_start(bp_t[:], b_proj[:])
        if with_bias_attn or with_bias_proj:
            ones_row = pool_c.tile([1, 512], BF16, tag="ones_row")
            nc.gpsimd.memset(ones_row[:], 1.0)

        # resident tensors
        pool_xT = ctx.enter_context(tc.tile_pool(name="xT", bufs=1))
        xT_sb = [pool_xT.tile([P, T], BF16, tag=f"xT{j}", name=f"xT{j}")
                 for j in range(CIN)]
        pool_v = ctx.enter_context(tc.tile_pool(name="vres", bufs=1))
        v_t = [pool_v.tile([P, H, D + 1], BF16, tag=f"v{i}", name=f"v{i}")
               for i in range(TKC)]
        pool_y = ctx.enter_context(tc.tile_pool(name="yres", bufs=1))
        yT_sb = [pool_y.tile([P, T], BF16, tag=f"y{j}", name=f"y{j}")
                 for j in range(CIN)]

        # working pools
        pool_qk = ctx.enter_context(tc.tile_pool(name="qkpool", bufs=8))
        pool_wqk = ctx.enter_context(tc.tile_pool(name="wqk", bufs=2))
        pool_wv = ctx.enter_context(tc.tile_pool(name="wv", bufs=1))
        pool_ex = ctx.enter_context(tc.tile_pool(name="expool", bufs=4))
        pool_yn = ctx.enter_context(tc.tile_pool(name="ynpool", bufs=6))
        pool_rc = ctx.enter_context(tc.tile_pool(name="rcpool", bufs=8))
        pool_ost = ctx.enter_context(tc.tile_pool(name="ostpool", bufs=4))
        pool_part = ctx.enter_context(tc.tile_pool(name="partpool", bufs=32))

        # PSUM: sT 2x2 banks + y 2x1 + mm 2x1 = 8 banks exactly
        psum_s = ctx.enter_context(
            tc.tile_pool(name="ps_s", bufs=2, space="PSUM"))
        psum_y = ctx.enter_context(
            tc.tile_pool(name="ps_y", bufs=2, space="PSUM"))
        psum_mm = ctx.enter_context(
            tc.tile_pool(name="ps_mm", bufs=2, space="PSUM"))

        for i in range(TKC):
            nc.gpsimd.memset(v_t[i][:, :, D:D + 1], 1.0)
        for j in range(CIN):
            nc.sync.dma_start(xT_sb[j][:], xT_in[j * P:(j + 1) * P, :])

        qkT = {}        # pr -> (qT tile, kT tile)
        parts = []      # staged proj half-0 partial sums (FIFO)
        state = {"v_prog": {0: -1, 1: -1}, "qk_prog": {}, "tr7": 0,
                 "tr": {}, "cur_pair": -1}

        # ---------------- feeder generators ----------------
        def gen_qkv(pr):
            # qT_pr reuses qT_{pr-4}'s SBUF slot (bufs=8), whose last reader
            # is attention(pr-4)'s S matmul: emitting this pair's copies
            # before attention(pr-2) starts can cycle the in-order DVE queue
            # against PE (observed deadlock) — gate on attention progress.
            while state["cur_pair"] < pr - 2:
                yield None
            qt = pool_qk.tile([P, T], BF16, tag="qk", name=f"qT{pr}")
            kt = pool_qk.tile([P, T], BF16, tag="qk", name=f"kT{pr}")
            qkT[pr] = (qt, kt)
            state["qk_prog"][pr] = -1
            wms = {}
            for m in (pr, PAIRS + pr):
                wm = pool_wqk.tile([P, CIN, P], BF16, tag="wqk", name="wm")
                nc.sync.dma_start(
                    wm[:],
                    wqk_in[m * P:(m + 1) * P, :].rearrange(
                        "p (j n) -> p j n", n=P))
                wms[m] = wm
            # q/k interleaved per 512-wide t-slice so attention(pr) qb j can
            # start as soon as slices <= j exist
            for tt in range(T // 512):
                for m, dst in ((pr, qt), (PAIRS + pr, kt)):
                    ps = psum_mm.tile([P, 512], F32, tag="mm", name="ps_qk")
                    for j in range(CIN):
                        nc.tensor.matmul(
                            ps[:], wms[m][:, j, :],
                            xT_sb[j][:, tt * 512:(tt + 1) * 512],
                            start=(j == 0),
                            stop=(j == CIN - 1 and not with_bias_attn),
                            skip_group_check=True)
                        if j == 3:
                            yield 0.9   # half-unit: finer fill pacing
                    if with_bias_attn:
                        col0 = m * P if m < PAIRS else C + (m - PAIRS) * P
                        nc.tensor.matmul(
                            ps[:], ba_t[0:1, col0:col0 + P],
                            ones_row[0:1, :], start=False, stop=True)
                    nc.vector.tensor_copy(
                        dst[:, tt * 512:(tt + 1) * 512], ps[:])
                    yield 0.9
                state["qk_prog"][pr] = tt

        def gen_v(slab):
            # slab 1 feeds pairs 4-7 only: hold its emission back so it can
            # fill the late pairs' exp-latency gaps instead of the early ones
            # (pair 4's per-qb barriers force what they need just in time)
            while slab == 1 and state["cur_pair"] < 3:
                yield None
            g = slab * 512
            wv_t = []
            for j in range(CIN):
                wv = pool_wv.tile([P, 512], BF16, tag=f"wv{j}", name=f"wv{j}")
                nc.sync.dma_start(
                    wv[:], wv_in[j * P:(j + 1) * P, g:g + 512])
                wv_t.append(wv)
            for i in range(TKC):
                ps = psum_mm.tile([P, 512], F32, tag="mm", name="ps_v")
                for j in range(CIN):
                    nc.tensor.matmul(
                        ps[:], xT_sb[j][:, i * P:(i + 1) * P], wv_t[j][:],
                        start=(j == 0),
                        stop=(j == CIN - 1 and not with_bias_attn),
                        skip_group_check=True)
                    if j == 3:
                        yield 0.9
                if with_bias_attn:
                    nc.tensor.matmul(
                        ps[:], ones_row[0:1, 0:P],
                        ba_t[0:1, 2 * C + g:2 * C + g + 512],
                        start=False, stop=True)
                nc.vector.tensor_copy(
                    v_t[i][:, g // D:(g + 512) // D, 0:D],
                    ps[:].rearrange("p (h d) -> p h d", d=D))
                state["v_prog"][slab] = i
                yield 0.9

        def gen_proj_half(h):
            # contraction split: half 0 (yT chunks 0-3) only needs pairs 0-3
            # and becomes PE fill for the otherwise-starved pairs 4-6; its
            # partial sums stage in SBUF (bf16) and half 1 adds them back.
            # Gate: pair h-half's transposes are all emitted once cur_pair
            # moves past the half's last pair (stage2 drains at pair end).
            while state["cur_pair"] < (3 if h == 0 else PAIRS - 1):
                yield None
            js = list(range(4 * h, 4 * h + 4))
            for g in (0, 512):
                wp_t = {}
                for j in js:
                    wp = pool_wv.tile([P, 512], BF16, tag=f"wp{j}",
                                      name=f"wp{j}")
                    nc.sync.dma_start(
                        wp[:], wp_in[j * P:(j + 1) * P, g:g + 512])
                    wp_t[j] = wp
                for i in range(TKC):
                    last_pr = 4 * h + 3
                    while (state["cur_pair"] <= last_pr and
                           state["tr"].get(last_pr, 0) < i + 1):
                        yield None
                    ps = psum_mm.tile([P, 512], F32, tag="mm", name="ps_o")
                    for j in js:
                        nc.tensor.matmul(
                            ps[:], yT_sb[j][:, i * P:(i + 1) * P], wp_t[j][:],
                            start=(j == js[0]),
                            stop=(j == js[-1] and not
                                  (h == 1 and with_bias_proj)),
                            skip_group_check=True)
                        if j == js[1]:
                            yield 0.45
                    if h == 0:
                        part = pool_part.tile([P, 512], BF16, tag="part",
                                              name="part")
                        nc.vector.tensor_copy(part[:], ps[:])
                        parts.append(part)
                    else:
                        if with_bias_proj:
                            nc.tensor.matmul(
                                ps[:], ones_row[0:1, 0:P],
                                bp_t[0:1, g:g + 512],
                                start=False, stop=True)
                        ost = pool_ost.tile([P, 512], F32, tag="ost",
                                            name="ost")
                        nc.vector.tensor_add(ost[:], ps[:],
                                             parts.pop(0)[:])
                        nc.sync.dma_start(
                            out_d[i * P:(i + 1) * P, g:g + 512], ost[:])
                    yield 0.5

        class Feeder:
            def __init__(self):
                self.gens = []

            def push(self, g):
                self.gens.append(g)

            def pull_one(self):
                """Advance one unit from the first non-blocked generator
                (blocked heads are skipped, order otherwise preserved).
                Returns cost (us), 0.0 if all blocked, None if exhausted."""
                idx = 0
                pl = state.get("pulls", {}).get(state["cur_pair"])
                while idx < len(self.gens):
                    try:
                        cost = next(self.gens[idx])
                    except StopIteration:
                        self.gens.pop(idx)
                        continue
                    if cost is None:
                        idx += 1        # gated — try the next generator
                        continue
                    if pl is not None:
                        pl[0] += cost
                        pl[1] += 1
                    return cost
                if pl is not None:
                    pl[2] += 1
                return None if not self.gens else 0.0

            def drain(self):
                while self.pull_one() is not None:
                    pass

        feeder = Feeder()

        # ---------------- attention ----------------
        def attention(pr):
            state["pulls"] = state.get("pulls", {})
            state["pulls"][pr] = [0.0, 0, 0]   # us pulled, n pulled, n blocked
            state["cur_pair"] = pr
            qT, kT = qkT[pr]
            slab = pr // 4
            pending1 = []   # stage1: normalize (DVE)
            pending2 = []   # stage2: transpose + yT copy (PE+DVE)

            def emit_stage1(y_e, y_o, s, i):
                rc_e = pool_rc.tile([P, 1], F32, tag="rc", name="rc_e")
                rc_o = pool_rc.tile([P, 1], F32, tag="rc", name="rc_o")
                yn = pool_yn.tile([P, P], F32, tag="yn", name="yn")
                with nc.allow_low_precision(reason="softmax normalize"):
                    nc.vector.reciprocal(rc_e[:], y_e[:, 65 * s + D:65 * s + D + 1])
                    nc.vector.reciprocal(rc_o[:], y_o[:, 65 * s + D:65 * s + D + 1])
                    nc.vector.tensor_scalar(
                        yn[:, 0:D], y_e[:, 65 * s:65 * s + D], rc_e[:, 0:1],
                        None, op0=mybir.AluOpType.mult)
                    nc.vector.tensor_scalar(
                        yn[:, D:2 * D], y_o[:, 65 * s:65 * s + D], rc_o[:, 0:1],
                        None, op0=mybir.AluOpType.mult)
                pending2.append((yn, i))

            def emit_stage2(yn, i):
                # PE transpose (DMA-engine transposes would head-block either
                # hwdge queue on the yn dependency: SP deadlocks vs wv/out
                # DMAs, ACT stalls exp dispatch)
                tr = psum_mm.tile([P, 512], F32, tag="mm", name="tr")
                nc.tensor.transpose(tr[:, 0:P], yn[:], ident_t[:])
                nc.vector.tensor_copy(yT_sb[pr][:, i * P:(i + 1) * P],
                                      tr[:, 0:P])
                state["tr"][pr] = i + 1
                if pr == PAIRS - 1:
                    state["tr7"] = i + 1

            debt = [0.0]

            def fill(extra=0.0):
                # in the last pair every pulled unit shortens the otherwise
                # ACT-idle projection tail: pull as hard as supply allows
                last = pr == PAIRS - 1
                debt[0] += extra + (0.25 if last else 0.0)
                pulls = 0
                while debt[0] > 0 and pulls < (4 if last else 2):
                    cost = feeder.pull_one()
                    if not cost:
                        break
                    debt[0] -= cost
                    pulls += 1
                debt[0] = min(debt[0], 8.0 if last else 4.0)

            for qb in range(NQB):
                q0 = qb * QB
                # this qb's q/k slices and v chunks must already be emitted
                spins = 0
                while (state["qk_prog"][pr] < qb or
                       state["v_prog"][slab] < min(4 * qb + 3, TKC - 1)):
                    c_ = feeder.pull_one()
                    spins += 1
                    if c_ is None or spins > 100000:
                        raise RuntimeError("feeder stuck before qk/v ready")
                y_e = psum_y.tile([P, 512], F32, tag="y", name="y_e")
                y_o = psum_y.tile([P, 512], F32, tag="y", name="y_o")

                def emit_pv(ex, n0, c, y_e=y_e, y_o=y_o, qb=qb):
                    s_min = max(0, c - 4 * qb)
                    for s in range(s_min, SUB):
                        # start=True clears has_written for the WHOLE bank, so
                        # only the tile's very first matmul may set it; later
                        # regions first-write via the cleared per-element bits
                        st_ = (c == 0 and s == 0)
                        sp_ = (c == 4 * qb + s)
                        nc.tensor.matmul(
                            y_e[:, 65 * s:65 * s + 65],
                            ex[:, s * P:(s + 1) * P],
                            v_t[c][:, 2 * pr, :],
                            start=st_, stop=sp_, skip_group_check=True)
                        nc.tensor.matmul(
                            y_o[:, 65 * s:65 * s + 65],
                            ex[:, QB + s * P - n0:QB + (s + 1) * P - n0],
                            v_t[c][:, 2 * pr + 1, :],
                            start=st_, stop=sp_, skip_group_check=True)
                        if sp_:
                            pending1.append((y_e, y_o, s, 4 * qb + s))

                pv_queue = []
                for c in range(4 * qb + 4):
                    n0 = max(0, c * P - q0)
                    sT = psum_s.tile([P, 2 * QB], F32, tag="sT", name="sT")
                    nc.tensor.matmul(
                        sT[:, n0:QB],
                        kT[0:D, c * P:(c + 1) * P],
                        qT[0:D, q0 + n0:q0 + QB],
                        start=True, stop=True, tile_position=(0, 0))
                    nc.tensor.matmul(
                        sT[:, QB:2 * QB - n0],
                        kT[D:2 * D, c * P:(c + 1) * P],
                        qT[D:2 * D, q0 + n0:q0 + QB],
                        start=True, stop=True, tile_position=(D, 0))
                    if pending2:
                        emit_stage2(*pending2.pop(0))
                    if pending1:
                        emit_stage1(*pending1.pop(0))
                    ex = pool_ex.tile([P, 2 * QB], BF16, tag="ex", name="ex")
                    nc.scalar.activation(ex[:, n0:2 * QB - n0],
                                         sT[:, n0:2 * QB - n0],
                                         mybir.ActivationFunctionType.Exp,
                                         scale=scale)
                    if c * P >= q0:   # diagonal 128-block: causal 0/1 mask
                        nc.vector.tensor_mul(ex[:, n0:n0 + P],
                                             ex[:, n0:n0 + P], mask_t[:])
                        nc.vector.tensor_mul(ex[:, QB:QB + P],
                                             ex[:, QB:QB + P], mask_t[:])
                    # debt-paced feeder fill: keep PE busy while ACT exps
                    s_min = max(0, c - 4 * qb)
                    act_c = (2 * QB - 2 * n0) * _ACT_US + 0.30
                    pe_c = (2 * (QB - n0) +
                            2 * (SUB - s_min) * 65) * _MM_US + 0.06
                    fill(act_c - pe_c)
                    # software pipeline (depth 2): PV lags the S/exp front by
                    # two chunks so the PE never couples to exp completion
                    pv_queue.append((ex, n0, c))
                    if len(pv_queue) > 1:
                        emit_pv(*pv_queue.pop(0))
                for it in pv_queue:
                    emit_pv(*it)
                # the y_e/y_o PSUM slots are recycled by the next qb's
                # allocation: every pending normalize reading them must be
                # emitted before that (stage2 may stay pending)
                while pending1:
                    emit_stage1(*pending1.pop(0))
                    feeder.pull_one()
            # drain transposes, interleaving feeder units
            while pending2:
                emit_stage2(*pending2.pop(0))
                feeder.pull_one()

        # ---------------- program ----------------
        # lead-in: pair 0's first q/k slice + first 4 v chunks directly
        g0 = gen_qkv(0)
        while state["qk_prog"].get(0, -1) < 0:
            next(g0)
        gv0 = gen_v(0)
        while state["v_prog"][0] < SUB - 1:
            next(gv0)
        feeder.push(g0)
        feeder.push(gv0)
        feeder.push(gen_qkv(1))
        feeder.push(gen_qkv(2))
        feeder.push(gen_qkv(3))
        feeder.push(gen_v(1))
        feeder.push(gen_qkv(4))
        feeder.push(gen_qkv(5))
        feeder.push(gen_qkv(6))
        feeder.push(gen_qkv(7))
        feeder.push(gen_proj_half(0))
        feeder.push(gen_proj_half(1))

        for pr in range(PAIRS):
            # barrier: this pair's first q/k slice must be emitted
            spins = 0
            while state["qk_prog"].get(pr, -1) < 0:
                c_ = feeder.pull_one()
                spins += 1
                if c_ is None or spins > 100000:
                    raise RuntimeError("feeder stuck before qk ready")
            attention(pr)
        feeder.drain()
        import os
        if os.environ.get("FEED_DEBUG"):
            for k in sorted(state.get("pulls", {})):
                us, n, blocked = state["pulls"][k]
                print(f"pair {k}: pulled {us:.1f}us in {n} units, "
                      f"{blocked} dry pulls")

    nc.compile()
    return nc


def make_const_inputs():
    ident = np.eye(P, dtype=np.float32)
    # S^T diagonal block mask: valid iff tq_local >= tk_local
    mask = np.triu(np.ones((P, P), np.float32)).astype(BF)
    return ident, mask


def make_in_maps(inputs, n_cores=8):
    """Host-side marshalling: shard x over batch, convert to bf16, transpose
    x, pack w_attn's q|k chunks into contiguous lhsT tiles."""
    x = np.asarray(inputs["x"], dtype=np.float32)
    w_attn = np.asarray(inputs["w_attn"], dtype=np.float32)
    w_proj = np.asarray(inputs["w_proj"], dtype=np.float32)
    b_attn = np.asarray(inputs.get("b_attn", 0), dtype=np.float32)
    b_proj = np.asarray(inputs.get("b_proj", 0), dtype=np.float32)
    B, T, C = x.shape

    wqk = w_attn[:, :2 * C]        # [C, 2C]
    # chunk m tile [p, j*128+n] = w_attn[j*128+p, m*128+n]
    wqk_packed = np.ascontiguousarray(
        wqk.reshape(C // P, P, 2 * C // P, P)     # [j, p, m, n]
        .transpose(2, 1, 0, 3)                    # [m, p, j, n]
        .reshape(2 * C, C)).astype(BF)
    wv = np.ascontiguousarray(w_attn[:, 2 * C:]).astype(BF)
    wp = np.ascontiguousarray(w_proj).astype(BF)
    ident, mask = make_const_inputs()

    wba = bool(np.any(b_attn != 0))
    wbp = bool(np.any(b_proj != 0))
    in_maps = []
    for i in range(n_cores):
        m = {"xT": np.ascontiguousarray(x[i].T).astype(BF),
             "wqk": wqk_packed, "wv": wv, "wp": wp,
             "ident": ident, "mask": mask}
        if wba:
            m["b_attn"] = b_attn.reshape(1, -1).astype(BF)
        if wbp:
            m["b_proj"] = b_proj.reshape(1, -1).astype(BF)
        in_maps.append(m)
    return in_maps


_CACHE = {}


def _get_program(T, C, H, wba, wbp, n_cores):
    key = (T, C, H, wba, wbp, n_cores)
    if key not in _CACHE:
        _CACHE[key] = build_program(T=T, C=C, H=H, n_cores=n_cores,
                                    with_bias_attn=wba, with_bias_proj=wbp)
    return _CACHE[key]


def kernel(x, w_attn, b_attn, w_proj, b_proj):
    x = np.asarray(x, dtype=np.float32)
    B, T, C = x.shape
    H = 16
    n_cores = 8
    assert B == n_cores

    inputs = {"x": x, "w_attn": w_attn, "b_attn": b_attn,
              "w_proj": w_proj, "b_proj": b_proj}
    in_maps = make_in_maps(inputs, n_cores)
    wba = "b_attn" in in_maps[0]
    wbp = "b_proj" in in_maps[0]
    nc = _get_program(T, C, H, wba, wbp, n_cores)

    res = run_bass_kernel_spmd(nc, in_maps, list(range(n_cores)))
    return np.stack([res.results[i]["out"] for i in range(n_cores)], axis=0)


# revision 45
# speedup vs baseline: 1.0764x; 1.0224x over previous
"""Causal self-attention Trainium2 Bass kernel, data-parallel over 8 NeuronCores.

Problem (hardcoded): x [8, 2048, 1024] fp32; w_attn [1024, 3072]; b_attn [3072];
w_proj [1024, 1024]; b_proj [1024]. H=16 heads, D=64.

Sharding: batch (8) -> one sample per core. Weights replicated. All matmul
operands are bf16 (inputs converted host-side; fp32 PSUM accumulation), which
keeps every matmul at 1 cycle/row on the PE and halves SBUF/DMA footprints.

Host-side input marshalling per core: x is pre-transposed to xT [C, T] and
w_attn's q|k halves are pre-packed so each 128-channel chunk loads as one
contiguous [128, C] lhsT tile.

Per-core dataflow (fused single pipeline, no DRAM staging):
  - qT/kT [128ch, T] = w_chunk^T @ xT, kept in SBUF (pool of 8 chunks,
    produced just in time and freed after the pair's attention)
  - v [tk, h, 64+1] in SBUF; column 64 is 1.0 (rides the PV matmul to
    produce softmax denominators)
  - per pair of heads (even/odd share a 128-channel chunk), per 512-wide
    q-block, per 128-wide k-chunk c:
      S^T merged tile [128tk, 1024]: even head at cols [n0:512] (bank 0),
      odd shifted to [512:1024-n0] (bank 1) so one Exp instruction (scale
      folded) covers both heads with no garbage zone -> ex bf16;
      causal mask = 0/1 multiply on the two diagonal 128-blocks only
      PV: ex 128-col slices are the STATIONARY operand, v [128,65] the
      moving one -> y [tq, 65] accumulated over c in PSUM (free dim 65
      instead of 512: halves PV cycles vs the yT layout); only the very
      first matmul per y bank sets start=True (start clears has_written
      for the WHOLE bank; per-element bits handle later regions)
  - normalize: per-partition reciprocal of the denominator column +
    tensor_scalar multiply (no PE broadcast needed in [tq, d] layout)
  - y -> yT via PE transpose (f32, 128x128) feeding the output projection
  - out [T, C] = y @ w_proj via lhsT = yT chunks, split into two
    contraction halves: half 0 (pairs 0-3) runs during pairs 4-6 with its
    partial sums staged in SBUF (bf16), half 1 adds them back at the end

All non-attention matmul work (qkv, v, both proj halves) is emitted through
generators interleaved into the attention c-loops (debt-paced, with
deadlock-avoiding gates) so the in-order PE queue never drains while the
ACT engine runs Exp; PV is software-pipelined one k-chunk behind S/Exp.
Cost-model makespan ~502us vs 648us for the f32r phase-sequential baseline.
"""

import numpy as np
from contextlib import ExitStack

import ml_dtypes

import concourse.bacc as bacc
import concourse.tile as tile
from concourse import mybir
from concourse.bass_utils import run_bass_kernel_spmd

F32 = mybir.dt.float32
BF16 = mybir.dt.bfloat16
BF = ml_dtypes.bfloat16
P = 128

# cost-model cycle estimates (us) used only for emission pacing
_MM_US = 0.0004167      # PE cycle @2.4GHz
_ACT_US = 0.000833      # ACT cycle @1.2GHz


def build_program(T=2048, C=1024, H=16, n_cores=8,
                  with_bias_attn=False, with_bias_proj=False):
    D = C // H            # 64
    assert D == 64 and H % 2 == 0
    CIN = C // P          # 8 contraction chunks
    PAIRS = C // P        # 8 head pairs
    TKC = T // P          # 16 k-chunks
    QB = 512
    NQB = T // QB         # 4
    SUB = QB // P         # 4
    scale = 1.0 / float(np.sqrt(D))

    nc = bacc.Bacc("TRN2", target_bir_lowering=False, debug=False,
                   num_devices=n_cores)

    xT_in = nc.dram_tensor("xT", [C, T], BF16, kind="ExternalInput")
    wqk_in = nc.dram_tensor("wqk", [2 * C, C], BF16, kind="ExternalInput")
    wv_in = nc.dram_tensor("wv", [C, C], BF16, kind="ExternalInput")
    wp_in = nc.dram_tensor("wp", [C, C], BF16, kind="ExternalInput")
    ident_in = nc.dram_tensor("ident", [P, P], F32, kind="ExternalInput")
    mask_in = nc.dram_tensor("mask", [P, P], BF16, kind="ExternalInput")
    if with_bias_attn:
        b_attn = nc.dram_tensor("b_attn", [1, 3 * C], BF16,
                                kind="ExternalInput")
    if with_bias_proj:
        b_proj = nc.dram_tensor("b_proj", [1, C], BF16, kind="ExternalInput")
    out_d = nc.dram_tensor("out", [T, C], F32, kind="ExternalOutput")

    with tile.TileContext(nc) as tc, ExitStack() as ctx:
        ctx.enter_context(nc.allow_low_precision(reason="bf16 pipeline"))
        pool_c = ctx.enter_context(tc.tile_pool(name="const", bufs=1))
        ident_t = pool_c.tile([P, P], F32, tag="ident")
        mask_t = pool_c.tile([P, P], BF16, tag="mask")
        nc.sync.dma_start(ident_t[:], ident_in[:])
        nc.sync.dma_start(mask_t[:], mask_in[:])
        if with_bias_attn:
            ba_t = pool_c.tile([1, 3 * C], BF16, tag="ba")
            nc.sync.dma_start(ba_t[:], b_attn[:])
        if with_bias_proj:
            bp_t = pool_c.tile([1, C], BF16, tag="bp")
            nc.sync.dma_start(bp_t[:], b_proj[:])
        if with_bias_attn or with_bias_proj:
            ones_row = pool_c.tile([1, 512], BF16, tag="ones_row")
            nc.gpsimd.memset(ones_row[:], 1.0)

        # resident tensors
        pool_xT = ctx.enter_context(tc.tile_pool(name="xT", bufs=1))
        xT_sb = [pool_xT.tile([P, T], BF16, tag=f"xT{j}", name=f"xT{j}")
                 for j in range(CIN)]
        pool_v = ctx.enter_context(tc.tile_pool(name="vres", bufs=1))
        v_t = [pool_v.tile([P, H, D + 1], BF16, tag=f"v{i}", name=f"v{i}")
               for i in range(TKC)]
        pool_y = ctx.enter_context(tc.tile_pool(name="yres", bufs=1))
        yT_sb = [pool_y.tile([P, T], BF16, tag=f"y{j}", name=f"y{j}")
                 for j in range(CIN)]

        # working pools
        pool_qk = ctx.enter_context(tc.tile_pool(name="qkpool", bufs=8))
        pool_wqk = ctx.enter_context(tc.tile_pool(name="wqk", bufs=2))
        pool_wv = ctx.enter_context(tc.tile_pool(name="wv", bufs=1))
        pool_ex = ctx.enter_context(tc.tile_pool(name="expool", bufs=4))
        pool_yn = ctx.enter_context(tc.tile_pool(name="ynpool", bufs=6))
        pool_rc = ctx.enter_context(tc.tile_pool(name="rcpool", bufs=8))
        pool_ost = ctx.enter_context(tc.tile_pool(name="ostpool", bufs=3))
        pool_part = ctx.enter_context(tc.tile_pool(name="partpool", bufs=32))

        # PSUM: sT 2x2 banks + y 2x1 + mm 2x1 = 8 banks exactly
        psum_s = ctx.enter_context(
            tc.tile_pool(name="ps_s", bufs=2, space="PSUM"))
        psum_y = ctx.enter_context(
            tc.tile_pool(name="ps_y", bufs=2, space="PSUM"))
        psum_mm = ctx.enter_context(
            tc.tile_pool(name="ps_mm", bufs=2, space="PSUM"))

        for i in range(TKC):
            nc.gpsimd.memset(v_t[i][:, :, D:D + 1], 1.0)
        for j in range(CIN):
            nc.sync.dma_start(xT_sb[j][:], xT_in[j * P:(j + 1) * P, :])

        qkT = {}        # pr -> (qT tile, kT tile)
        parts = []      # staged proj half-0 partial sums (FIFO)
        state = {"v_prog": {0: -1, 1: -1}, "qk_prog": {}, "tr7": 0,
                 "tr": {}, "cur_pair": -1}

        # ---------------- feeder generators ----------------
        def gen_qkv(pr):
            # qT_pr reuses qT_{pr-4}'s SBUF slot (bufs=8), whose last reader
            # is attention(pr-4)'s S matmul: emitting this pair's copies
            # before attention(pr-2) starts can cycle the in-order DVE queue
            # against PE (observed deadlock) — gate on attention progress.
            while state["cur_pair"] < pr - 2:
                yield None
            qt = pool_qk.tile([P, T], BF16, tag="qk", name=f"qT{pr}")
            kt = pool_qk.tile([P, T], BF16, tag="qk", name=f"kT{pr}")
            qkT[pr] = (qt, kt)
            state["qk_prog"][pr] = -1
            wms = {}
            for m in (pr, PAIRS + pr):
                wm = pool_wqk.tile([P, CIN, P], BF16, tag="wqk", name="wm")
                nc.sync.dma_start(
                    wm[:],
                    wqk_in[m * P:(m + 1) * P, :].rearrange(
                        "p (j n) -> p j n", n=P))
                wms[m] = wm
            # q/k interleaved per 512-wide t-slice so attention(pr) qb j can
            # start as soon as slices <= j exist
            for tt in range(T // 512):
                for m, dst in ((pr, qt), (PAIRS + pr, kt)):
                    ps = psum_mm.tile([P, 512], F32, tag="mm", name="ps_qk")
                    for j in range(CIN):
                        nc.tensor.matmul(
                            ps[:], wms[m][:, j, :],
                            xT_sb[j][:, tt * 512:(tt + 1) * 512],
                            start=(j == 0),
                            stop=(j == CIN - 1 and not with_bias_attn),
                            skip_group_check=True)
                        if j == 3:
                            yield 0.9   # half-unit: finer fill pacing
                    if with_bias_attn:
                        col0 = m * P if m < PAIRS else C + (m - PAIRS) * P
                        nc.tensor.matmul(
                            ps[:], ba_t[0:1, col0:col0 + P],
                            ones_row[0:1, :], start=False, stop=True)
                    nc.vector.tensor_copy(
                        dst[:, tt * 512:(tt + 1) * 512], ps[:])
                    yield 0.9
                state["qk_prog"][pr] = tt

        def gen_v(slab):
            # slab 1 feeds pairs 4-7 only: hold its emission back so it can
            # fill the late pairs' exp-latency gaps instead of the early ones
            # (pair 4's per-qb barriers force what they need just in time)
            while slab == 1 and state["cur_pair"] < 3:
                yield None
            g = slab * 512
            wv_t = []
            for j in range(CIN):
                wv = pool_wv.tile([P, 512], BF16, tag=f"wv{j}", name=f"wv{j}")
                nc.sync.dma_start(
                    wv[:], wv_in[j * P:(j + 1) * P, g:g + 512])
                wv_t.append(wv)
            for i in range(TKC):
                ps = psum_mm.tile([P, 512], F32, tag="mm", name="ps_v")
                for j in range(CIN):
                    nc.tensor.matmul(
                        ps[:], xT_sb[j][:, i * P:(i + 1) * P], wv_t[j][:],
                        start=(j == 0),
                        stop=(j == CIN - 1 and not with_bias_attn),
                        skip_group_check=True)
                    if j == 3:
                        yield 0.9
                if with_bias_attn:
                    nc.tensor.matmul(
                        ps[:], ones_row[0:1, 0:P],
                        ba_t[0:1, 2 * C + g:2 * C + g + 512],
                        start=False, stop=True)
                nc.vector.tensor_copy(
                    v_t[i][:, g // D:(g + 512) // D, 0:D],
                    ps[:].rearrange("p (h d) -> p h d", d=D))
                state["v_prog"][slab] = i
                yield 0.9

        def gen_proj_half(h):
            # contraction split: half 0 (yT chunks 0-3) only needs pairs 0-3
            # and becomes PE fill for the otherwise-starved pairs 4-6; its
            # partial sums stage in SBUF (bf16) and half 1 adds them back.
            # Gate: pair h-half's transposes are all emitted once cur_pair
            # moves past the half's last pair (stage2 drains at pair end).
            while state["cur_pair"] < (3 if h == 0 else PAIRS - 1):
                yield None
            js = list(range(4 * h, 4 * h + 4))
            for g in (0, 512):
                wp_t = {}
                for j in js:
                    wp = pool_wv.tile([P, 512], BF16, tag=f"wp{j}",
                                      name=f"wp{j}")
                    nc.sync.dma_start(
                        wp[:], wp_in[j * P:(j + 1) * P, g:g + 512])
                    wp_t[j] = wp
                for i in range(TKC):
                    last_pr = 4 * h + 3
                    while (state["cur_pair"] <= last_pr and
                           state["tr"].get(last_pr, 0) < i + 1):
                        yield None
                    ps = psum_mm.tile([P, 512], F32, tag="mm", name="ps_o")
                    for j in js:
                        nc.tensor.matmul(
                            ps[:], yT_sb[j][:, i * P:(i + 1) * P], wp_t[j][:],
                            start=(j == js[0]),
                            stop=(j == js[-1] and not
                                  (h == 1 and with_bias_proj)),
                            skip_group_check=True)
                        if j == js[1]:
                            yield 0.45
                    if h == 0:
                        part = pool_part.tile([P, 512], BF16, tag="part",
                                              name="part")
                        nc.vector.tensor_copy(part[:], ps[:])
                        parts.append(part)
                    else:
                        if with_bias_proj:
                            nc.tensor.matmul(
                                ps[:], ones_row[0:1, 0:P],
                                bp_t[0:1, g:g + 512],
                                start=False, stop=True)
                        ost = pool_ost.tile([P, 512], F32, tag="ost",
                                            name="ost")
                        nc.vector.tensor_add(ost[:], ps[:],
                                             parts.pop(0)[:])
                        nc.sync.dma_start(
                            out_d[i * P:(i + 1) * P, g:g + 512], ost[:])
                    yield 0.5

        class Feeder:
            def __init__(self):
                self.gens = []

            def push(self, g):
                self.gens.append(g)

            def pull_one(self):
                """Advance one unit from the first non-blocked generator
                (blocked heads are skipped, order otherwise preserved).
                Returns cost (us), 0.0 if all blocked, None if exhausted."""
                idx = 0
                pl = state.get("pulls", {}).get(state["cur_pair"])
                while idx < len(self.gens):
                    try:
                        cost = next(self.gens[idx])
                    except StopIteration:
                        self.gens.pop(idx)
                        continue
                    if cost is None:
                        idx += 1        # gated — try the next generator
                        continue
                    if pl is not None:
                        pl[0] += cost
                        pl[1] += 1
                    return cost
                if pl is not None:
                    pl[2] += 1
                return None if not self.gens else 0.0

            def drain(self):
                while self.pull_one() is not None:
                    pass

        feeder = Feeder()

        # ---------------- attention ----------------
        def attention(pr):
            state["pulls"] = state.get("pulls", {})
            state["pulls"][pr] = [0.0, 0, 0]   # us pulled, n pulled, n blocked
            state["cur_pair"] = pr
            qT, kT = qkT[pr]
            slab = pr // 4
            pending1 = []   # stage1: normalize (DVE)
            pending2 = []   # stage2: transpose + yT copy (PE+DVE)

            def emit_stage1(y_e, y_o, s, i):
                rc_e = pool_rc.tile([P, 1], F32, tag="rc", name="rc_e")
                rc_o = pool_rc.tile([P, 1], F32, tag="rc", name="rc_o")
                yn = pool_yn.tile([P, P], F32, tag="yn", name="yn")
                with nc.allow_low_precision(reason="softmax normalize"):
                    nc.vector.reciprocal(rc_e[:], y_e[:, 65 * s + D:65 * s + D + 1])
                    nc.vector.reciprocal(rc_o[:], y_o[:, 65 * s + D:65 * s + D + 1])
                    nc.vector.tensor_scalar(
                        yn[:, 0:D], y_e[:, 65 * s:65 * s + D], rc_e[:, 0:1],
                        None, op0=mybir.AluOpType.mult)
                    nc.vector.tensor_scalar(
                        yn[:, D:2 * D], y_o[:, 65 * s:65 * s + D], rc_o[:, 0:1],
                        None, op0=mybir.AluOpType.mult)
                pending2.append((yn, i))

            def emit_stage2(yn, i):
                # PE transpose (DMA-engine transposes would head-block either
                # hwdge queue on the yn dependency: SP deadlocks vs wv/out
                # DMAs, ACT stalls exp dispatch)
                tr = psum_mm.tile([P, 512], F32, tag="mm", name="tr")
                nc.tensor.transpose(tr[:, 0:P], yn[:], ident_t[:])
                nc.vector.tensor_copy(yT_sb[pr][:, i * P:(i + 1) * P],
                                      tr[:, 0:P])
                state["tr"][pr] = i + 1
                if pr == PAIRS - 1:
                    state["tr7"] = i + 1

            debt = [0.0]

            def fill(extra=0.0):
                # in the last pair every pulled unit shortens the otherwise
                # ACT-idle projection tail: pull as hard as supply allows
                last = pr == PAIRS - 1
                debt[0] += extra + (0.25 if last else 0.0)
                pulls = 0
                while debt[0] > 0 and pulls < (4 if last else 2):
                    cost = feeder.pull_one()
                    if not cost:
                        break
                    debt[0] -= cost
                    pulls += 1
                debt[0] = min(debt[0], 8.0 if last else 4.0)

            for qb in range(NQB):
                q0 = qb * QB
                # this qb's q/k slices and v chunks must already be emitted
                spins = 0
                while (state["qk_prog"][pr] < qb or
                       state["v_prog"][slab] < min(4 * qb + 3, TKC - 1)):
                    c_ = feeder.pull_one()
                    spins += 1
                    if c_ is None or spins > 100000:
                        raise RuntimeError("feeder stuck before qk/v ready")
                y_e = psum_y.tile([P, 512], F32, tag="y", name="y_e")
                y_o = psum_y.tile([P, 512], F32, tag="y", name="y_o")

                def emit_pv(ex, n0, c, y_e=y_e, y_o=y_o, qb=qb):
                    s_min = max(0, c - 4 * qb)
                    for s in range(s_min, SUB):
                        # start=True clears has_written for the WHOLE bank, so
                        # only the tile's very first matmul may set it; later
                        # regions first-write via the cleared per-element bits
                        st_ = (c == 0 and s == 0)
                        sp_ = (c == 4 * qb + s)
                        nc.tensor.matmul(
                            y_e[:, 65 * s:65 * s + 65],
                            ex[:, s * P:(s + 1) * P],
                            v_t[c][:, 2 * pr, :],
                            start=st_, stop=sp_, skip_group_check=True)
                        nc.tensor.matmul(
                            y_o[:, 65 * s:65 * s + 65],
                            ex[:, QB + s * P - n0:QB + (s + 1) * P - n0],
                            v_t[c][:, 2 * pr + 1, :],
                            start=st_, stop=sp_, skip_group_check=True)
                        if sp_:
                            pending1.append((y_e, y_o, s, 4 * qb + s))

                pv_queue = []
                for c in range(4 * qb + 4):
                    n0 = max(0, c * P - q0)
                    sT = psum_s.tile([P, 2 * QB], F32, tag="sT", name="sT")
                    nc.tensor.matmul(
                        sT[:, n0:QB],
                        kT[0:D, c * P:(c + 1) * P],
                        qT[0:D, q0 + n0:q0 + QB],
                        start=True, stop=True, tile_position=(0, 0))
                    nc.tensor.matmul(
                        sT[:, QB:2 * QB - n0],
                        kT[D:2 * D, c * P:(c + 1) * P],
                        qT[D:2 * D, q0 + n0:q0 + QB],
                        start=True, stop=True, tile_position=(D, 0))
                    if pending2:
                        emit_stage2(*pending2.pop(0))
                    if pending1:
                        emit_stage1(*pending1.pop(0))
                    ex = pool_ex.tile([P, 2 * QB], BF16, tag="ex", name="ex")
                    nc.scalar.activation(ex[:, n0:2 * QB - n0],
                                         sT[:, n0:2 * QB - n0],
                                         mybir.ActivationFunctionType.Exp,
                                         scale=scale)
                    if c * P >= q0:   # diagonal 128-block: causal 0/1 mask
                        nc.vector.tensor_mul(ex[:, n0:n0 + P],
                                             ex[:, n0:n0 + P], mask_t[:])
                        nc.vector.tensor_mul(ex[:, QB:QB + P],
                                             ex[:, QB:QB + P], mask_t[:])
                    # debt-paced feeder fill: keep PE busy while ACT exps
                    s_min = max(0, c - 4 * qb)
                    act_c = (2 * QB - 2 * n0) * _ACT_US + 0.30
                    pe_c = (2 * (QB - n0) +
                            2 * (SUB - s_min) * 65) * _MM_US + 0.06
                    fill(act_c - pe_c)
                    # software pipeline (depth 2): PV lags the S/exp front by
                    # two chunks so the PE never couples to exp completion
                    pv_queue.append((ex, n0, c))
                    if len(pv_queue) > 1:
                        emit_pv(*pv_queue.pop(0))
                for it in pv_queue:
                    emit_pv(*it)
                # the y_e/y_o PSUM slots are recycled by the next qb's
                # allocation: every pending normalize reading them must be
                # emitted before that (stage2 may stay pending)
                while pending1:
                    emit_stage1(*pending1.pop(0))
                    feeder.pull_one()
            # drain transposes, interleaving feeder units
            while pending2:
                emit_stage2(*pending2.pop(0))
                feeder.pull_one()

        # ---------------- program ----------------
        # lead-in: pair 0's first q/k slice + first 4 v chunks directly
        g0 = gen_qkv(0)
        while state["qk_prog"].get(0, -1) < 0:
            next(g0)
        gv0 = gen_v(0)
        while state["v_prog"][0] < SUB - 1:
            next(gv0)
        feeder.push(g0)
        feeder.push(gv0)
        feeder.push(gen_qkv(1))
        feeder.push(gen_qkv(2))
        feeder.push(gen_qkv(3))
        feeder.push(gen_v(1))
        feeder.push(gen_qkv(4))
        feeder.push(gen_qkv(5))
        feeder.push(gen_qkv(6))
        feeder.push(gen_qkv(7))
        feeder.push(gen_proj_half(0))
        feeder.push(gen_proj_half(1))

        for pr in range(PAIRS):
            # barrier: this pair's first q/k slice must be emitted
            spins = 0
            while state["qk_prog"].get(pr, -1) < 0:
                c_ = feeder.pull_one()
                spins += 1
                if c_ is None or spins > 100000:
                    raise RuntimeError("feeder stuck before qk ready")
            attention(pr)
        feeder.drain()
        import os
        if os.environ.get("FEED_DEBUG"):
            for k in sorted(state.get("pulls", {})):
                us, n, blocked = state["pulls"][k]
                print(f"pair {k}: pulled {us:.1f}us in {n} units, "
                      f"{blocked} dry pulls")

    nc.compile()
    return nc


def make_const_inputs():
    ident = np.eye(P, dtype=np.float32)
    # S^T diagonal block mask: valid iff tq_local >= tk_local
    mask = np.triu(np.ones((P, P), np.float32)).astype(BF)
    return ident, mask


def make_in_maps(inputs, n_cores=8):
    """Host-side marshalling: shard x over batch, convert to bf16, transpose
    x, pack w_attn's q|k chunks into contiguous lhsT tiles."""
    x = np.asarray(inputs["x"], dtype=np.float32)
    w_attn = np.asarray(inputs["w_attn"], dtype=np.float32)
    w_proj = np.asarray(inputs["w_proj"], dtype=np.float32)
    b_attn = np.asarray(inputs.get("b_attn", 0), dtype=np.float32)
    b_proj = np.asarray(inputs.get("b_proj", 0), dtype=np.float32)
    B, T, C = x.shape

    wqk = w_attn[:, :2 * C]        # [C, 2C]
    # chunk m tile [p, j*128+n] = w_attn[j*128+p, m*128+n]
    wqk_packed = np.ascontiguousarray(
        wqk.reshape(C // P, P, 2 * C // P, P)     # [j, p, m, n]
        .transpose(2, 1, 0, 3)                    # [m, p, j, n]
        .reshape(2 * C, C)).astype(BF)
    wv = np.ascontiguousarray(w_attn[:, 2 * C:]).astype(BF)
    wp = np.ascontiguousarray(w_proj).astype(BF)
    ident, mask = make_const_inputs()

    wba = bool(np.any(b_attn != 0))
    wbp = bool(np.any(b_proj != 0))
    in_maps = []
    for i in range(n_cores):
        m = {"xT": np.ascontiguousarray(x[i].T).astype(BF),
             "wqk": wqk_packed, "wv": wv, "wp": wp,
             "ident": ident, "mask": mask}
        if wba:
            m["b_attn"] = b_attn.reshape(1, -1).astype(BF)
        if wbp:
            m["b_proj"] = b_proj.reshape(1, -1).astype(BF)
        in_maps.append(m)
    return in_maps


_CACHE = {}


def _get_program(T, C, H, wba, wbp, n_cores):
    key = (T, C, H, wba, wbp, n_cores)
    if key not in _CACHE:
        _CACHE[key] = build_program(T=T, C=C, H=H, n_cores=n_cores,
                                    with_bias_attn=wba, with_bias_proj=wbp)
    return _CACHE[key]


def kernel(x, w_attn, b_attn, w_proj, b_proj):
    x = np.asarray(x, dtype=np.float32)
    B, T, C = x.shape
    H = 16
    n_cores = 8
    assert B == n_cores

    inputs = {"x": x, "w_attn": w_attn, "b_attn": b_attn,
              "w_proj": w_proj, "b_proj": b_proj}
    in_maps = make_in_maps(inputs, n_cores)
    wba = "b_attn" in in_maps[0]
    wbp = "b_proj" in in_maps[0]
    nc = _get_program(T, C, H, wba, wbp, n_cores)

    res = run_bass_kernel_spmd(nc, in_maps, list(range(n_cores)))
    return np.stack([res.results[i]["out"] for i in range(n_cores)], axis=0)


# revision 50
# speedup vs baseline: 1.0843x; 1.0074x over previous
"""Causal self-attention Trainium2 Bass kernel, data-parallel over 8 NeuronCores.

Problem (hardcoded): x [8, 2048, 1024] fp32; w_attn [1024, 3072]; b_attn [3072];
w_proj [1024, 1024]; b_proj [1024]. H=16 heads, D=64.

Sharding: batch (8) -> one sample per core. Weights replicated. All matmul
operands are bf16 (inputs converted host-side; fp32 PSUM accumulation), which
keeps every matmul at 1 cycle/row on the PE and halves SBUF/DMA footprints.

Host-side input marshalling per core: x is pre-transposed to xT [C, T] and
w_attn's q|k halves are pre-packed so each 128-channel chunk loads as one
contiguous [128, C] lhsT tile.

Per-core dataflow (fused single pipeline, no DRAM staging):
  - qT/kT [128ch, T] = w_chunk^T @ xT, kept in SBUF (pool of 8 chunks,
    produced just in time and freed after the pair's attention)
  - v [tk, h, 64+1] in SBUF; column 64 is 1.0 (rides the PV matmul to
    produce softmax denominators)
  - per pair of heads (even/odd share a 128-channel chunk), per 512-wide
    q-block, per 128-wide k-chunk c:
      S^T merged tile [128tk, 1024]: even head at cols [n0:512] (bank 0),
      odd shifted to [512:1024-n0] (bank 1) so one Exp instruction (scale
      folded) covers both heads with no garbage zone -> ex bf16;
      causal mask = 0/1 multiply on the two diagonal 128-blocks only
      PV: ex 128-col slices are the STATIONARY operand, v [128,65] the
      moving one -> y [tq, 65] accumulated over c in PSUM (free dim 65
      instead of 512: halves PV cycles vs the yT layout); only the very
      first matmul per y bank sets start=True (start clears has_written
      for the WHOLE bank; per-element bits handle later regions)
  - normalize: per-partition reciprocal of the denominator column +
    tensor_scalar multiply (no PE broadcast needed in [tq, d] layout)
  - y -> yT via PE transpose (f32, 128x128) feeding the output projection
  - out [T, C] = y @ w_proj via lhsT = yT chunks, split into two
    contraction halves: half 0 (pairs 0-3) runs during pairs 4-6 with its
    partial sums staged in SBUF (bf16), half 1 adds them back at the end

All non-attention matmul work (qkv, v, both proj halves) is emitted through
generators interleaved into the attention c-loops (debt-paced, with
deadlock-avoiding gates) so the in-order PE queue never drains while the
ACT engine runs Exp; PV is software-pipelined one k-chunk behind S/Exp.
Cost-model makespan ~502us vs 648us for the f32r phase-sequential baseline.
"""

import numpy as np
from contextlib import ExitStack

import ml_dtypes

import concourse.bacc as bacc
import concourse.tile as tile
from concourse import mybir
from concourse.bass_utils import run_bass_kernel_spmd

F32 = mybir.dt.float32
BF16 = mybir.dt.bfloat16
BF = ml_dtypes.bfloat16
P = 128

# cost-model cycle estimates (us) used only for emission pacing
_MM_US = 0.0004167      # PE cycle @2.4GHz
_ACT_US = 0.000833      # ACT cycle @1.2GHz


def build_program(T=2048, C=1024, H=16, n_cores=8,
                  with_bias_attn=False, with_bias_proj=False):
    D = C // H            # 64
    assert D == 64 and H % 2 == 0
    CIN = C // P          # 8 contraction chunks
    PAIRS = C // P        # 8 head pairs
    TKC = T // P          # 16 k-chunks
    QB = 512
    NQB = T // QB         # 4
    SUB = QB // P         # 4
    scale = 1.0 / float(np.sqrt(D))

    nc = bacc.Bacc("TRN2", target_bir_lowering=False, debug=False,
                   num_devices=n_cores)

    xT_in = nc.dram_tensor("xT", [C, T], BF16, kind="ExternalInput")
    wqk_in = nc.dram_tensor("wqk", [2 * C, C], BF16, kind="ExternalInput")
    wv_in = nc.dram_tensor("wv", [C, C], BF16, kind="ExternalInput")
    wp_in = nc.dram_tensor("wp", [C, C], BF16, kind="ExternalInput")
    ident_in = nc.dram_tensor("ident", [P, P], F32, kind="ExternalInput")
    identb_in = nc.dram_tensor("identb", [P, P], BF16, kind="ExternalInput")
    mask_in = nc.dram_tensor("mask", [P, P], BF16, kind="ExternalInput")
    if with_bias_attn:
        b_attn = nc.dram_tensor("b_attn", [1, 3 * C], BF16,
                                kind="ExternalInput")
    if with_bias_proj:
        b_proj = nc.dram_tensor("b_proj", [1, C], BF16, kind="ExternalInput")
    out_d = nc.dram_tensor("out", [T, C], F32, kind="ExternalOutput")

    with tile.TileContext(nc) as tc, ExitStack() as ctx:
        ctx.enter_context(nc.allow_low_precision(reason="bf16 pipeline"))
        pool_c = ctx.enter_context(tc.tile_pool(name="const", bufs=1))
        ident_t = pool_c.tile([P, P], F32, tag="ident")
        identb_t = pool_c.tile([P, P], BF16, tag="identb")
        mask_t = pool_c.tile([P, P], BF16, tag="mask")
        nc.sync.dma_start(ident_t[:], ident_in[:])
        nc.sync.dma_start(identb_t[:], identb_in[:])
        nc.sync.dma_start(mask_t[:], mask_in[:])
        if with_bias_attn:
            ba_t = pool_c.tile([1, 3 * C], BF16, tag="ba")
            nc.sync.dma_start(ba_t[:], b_attn[:])
        if with_bias_proj:
            bp_t = pool_c.tile([1, C], BF16, tag="bp")
            nc.sync.dma_start(bp_t[:], b_proj[:])
        if with_bias_attn or with_bias_proj:
            ones_row = pool_c.tile([1, 512], BF16, tag="ones_row")
            nc.gpsimd.memset(ones_row[:], 1.0)

        # resident tensors
        pool_xT = ctx.enter_context(tc.tile_pool(name="xT", bufs=1))
        xT_sb = [pool_xT.tile([P, T], BF16, tag=f"xT{j}", name=f"xT{j}")
                 for j in range(CIN)]
        pool_v = ctx.enter_context(tc.tile_pool(name="vres", bufs=1))
        v_t = [pool_v.tile([P, H, D + 1], BF16, tag=f"v{i}", name=f"v{i}")
               for i in range(TKC)]
        pool_y = ctx.enter_context(tc.tile_pool(name="yres", bufs=1))
        yT_sb = [pool_y.tile([P, T], BF16, tag=f"y{j}", name=f"y{j}")
                 for j in range(CIN)]

        # working pools
        pool_qk = ctx.enter_context(tc.tile_pool(name="qkpool", bufs=8))
        pool_wqk = ctx.enter_context(tc.tile_pool(name="wqk", bufs=2))
        pool_wv = ctx.enter_context(tc.tile_pool(name="wv", bufs=1))
        pool_ex = ctx.enter_context(tc.tile_pool(name="expool", bufs=4))
        pool_yn = ctx.enter_context(tc.tile_pool(name="ynpool", bufs=6))
        pool_rc = ctx.enter_context(tc.tile_pool(name="rcpool", bufs=8))
        pool_ost = ctx.enter_context(tc.tile_pool(name="ostpool", bufs=3))
        pool_part = ctx.enter_context(tc.tile_pool(name="partpool", bufs=32))

        # PSUM: sT 2x2 banks + y 2x1 + mm 2x1 = 8 banks exactly
        psum_s = ctx.enter_context(
            tc.tile_pool(name="ps_s", bufs=2, space="PSUM"))
        psum_y = ctx.enter_context(
            tc.tile_pool(name="ps_y", bufs=2, space="PSUM"))
        psum_mm = ctx.enter_context(
            tc.tile_pool(name="ps_mm", bufs=2, space="PSUM"))

        for i in range(TKC):
            nc.gpsimd.memset(v_t[i][:, :, D:D + 1], 1.0)
        for j in range(CIN):
            nc.sync.dma_start(xT_sb[j][:], xT_in[j * P:(j + 1) * P, :])

        qkT = {}        # pr -> (qT tile, kT tile)
        parts = []      # staged proj half-0 partial sums (FIFO)
        state = {"v_prog": {0: -1, 1: -1}, "qk_prog": {}, "tr7": 0,
                 "tr": {}, "cur_pair": -1}

        # ---------------- feeder generators ----------------
        def gen_qkv(pr):
            # qT_pr reuses qT_{pr-4}'s SBUF slot (bufs=8), whose last reader
            # is attention(pr-4)'s S matmul: emitting this pair's copies
            # before attention(pr-2) starts can cycle the in-order DVE queue
            # against PE (observed deadlock) — gate on attention progress.
            while state["cur_pair"] < pr - 2:
                yield None
            qt = pool_qk.tile([P, T], BF16, tag="qk", name=f"qT{pr}")
            kt = pool_qk.tile([P, T], BF16, tag="qk", name=f"kT{pr}")
            qkT[pr] = (qt, kt)
            state["qk_prog"][pr] = -1
            wms = {}
            for m in (pr, PAIRS + pr):
                wm = pool_wqk.tile([P, CIN, P], BF16, tag="wqk", name="wm")
                nc.sync.dma_start(
                    wm[:],
                    wqk_in[m * P:(m + 1) * P, :].rearrange(
                        "p (j n) -> p j n", n=P))
                wms[m] = wm
            # q/k interleaved per 512-wide t-slice so attention(pr) qb j can
            # start as soon as slices <= j exist
            for tt in range(T // 512):
                for m, dst in ((pr, qt), (PAIRS + pr, kt)):
                    ps = psum_mm.tile([P, 512], F32, tag="mm", name="ps_qk")
                    for j in range(CIN):
                        nc.tensor.matmul(
                            ps[:], wms[m][:, j, :],
                            xT_sb[j][:, tt * 512:(tt + 1) * 512],
                            start=(j == 0),
                            stop=(j == CIN - 1 and not with_bias_attn),
                            skip_group_check=True)
                        if j == 3:
                            yield 0.9   # half-unit: finer fill pacing
                    if with_bias_attn:
                        col0 = m * P if m < PAIRS else C + (m - PAIRS) * P
                        nc.tensor.matmul(
                            ps[:], ba_t[0:1, col0:col0 + P],
                            ones_row[0:1, :], start=False, stop=True)
                    nc.vector.tensor_copy(
                        dst[:, tt * 512:(tt + 1) * 512], ps[:])
                    yield 0.9
                state["qk_prog"][pr] = tt

        def gen_v(slab):
            # slab 1 feeds pairs 4-7 only: hold its emission back so it can
            # fill the late pairs' exp-latency gaps instead of the early ones
            # (pair 4's per-qb barriers force what they need just in time)
            while slab == 1 and state["cur_pair"] < 3:
                yield None
            g = slab * 512
            wv_t = []
            for j in range(CIN):
                wv = pool_wv.tile([P, 512], BF16, tag=f"wv{j}", name=f"wv{j}")
                nc.sync.dma_start(
                    wv[:], wv_in[j * P:(j + 1) * P, g:g + 512])
                wv_t.append(wv)
            for i in range(TKC):
                ps = psum_mm.tile([P, 512], F32, tag="mm", name="ps_v")
                for j in range(CIN):
                    nc.tensor.matmul(
                        ps[:], xT_sb[j][:, i * P:(i + 1) * P], wv_t[j][:],
                        start=(j == 0),
                        stop=(j == CIN - 1 and not with_bias_attn),
                        skip_group_check=True)
                    if j == 3:
                        yield 0.9
                if with_bias_attn:
                    nc.tensor.matmul(
                        ps[:], ones_row[0:1, 0:P],
                        ba_t[0:1, 2 * C + g:2 * C + g + 512],
                        start=False, stop=True)
                nc.vector.tensor_copy(
                    v_t[i][:, g // D:(g + 512) // D, 0:D],
                    ps[:].rearrange("p (h d) -> p h d", d=D))
                state["v_prog"][slab] = i
                yield 0.9

        def gen_proj_half(h):
            # contraction split: half 0 (yT chunks 0-3) only needs pairs 0-3
            # and becomes PE fill for the otherwise-starved pairs 4-6; its
            # partial sums stage in SBUF (bf16) and half 1 adds them back.
            # Gate: pair h-half's transposes are all emitted once cur_pair
            # moves past the half's last pair (stage2 drains at pair end).
            while state["cur_pair"] < (3 if h == 0 else PAIRS - 1):
                yield None
            js = list(range(4 * h, 4 * h + 4))
            for g in (0, 512):
                wp_t = {}
                for j in js:
                    wp = pool_wv.tile([P, 512], BF16, tag=f"wp{j}",
                                      name=f"wp{j}")
                    nc.sync.dma_start(
                        wp[:], wp_in[j * P:(j + 1) * P, g:g + 512])
                    wp_t[j] = wp
                for i in range(TKC):
                    last_pr = 4 * h + 3
                    while (state["cur_pair"] <= last_pr and
                           state["tr"].get(last_pr, 0) < i + 1):
                        yield None
                    ps = psum_mm.tile([P, 512], F32, tag="mm", name="ps_o")
                    for j in js:
                        nc.tensor.matmul(
                            ps[:], yT_sb[j][:, i * P:(i + 1) * P], wp_t[j][:],
                            start=(j == js[0]),
                            stop=(j == js[-1] and not
                                  (h == 1 and with_bias_proj)),
                            skip_group_check=True)
                        if j == js[1]:
                            yield 0.45
                    if h == 0:
                        part = pool_part.tile([P, 512], BF16, tag="part",
                                              name="part")
                        nc.vector.tensor_copy(part[:], ps[:])
                        parts.append(part)
                    else:
                        if with_bias_proj:
                            nc.tensor.matmul(
                                ps[:], ones_row[0:1, 0:P],
                                bp_t[0:1, g:g + 512],
                                start=False, stop=True)
                        ost = pool_ost.tile([P, 512], F32, tag="ost",
                                            name="ost")
                        nc.vector.tensor_add(ost[:], ps[:],
                                             parts.pop(0)[:])
                        nc.sync.dma_start(
                            out_d[i * P:(i + 1) * P, g:g + 512], ost[:])
                    yield 0.5

        class Feeder:
            def __init__(self):
                self.gens = []

            def push(self, g):
                self.gens.append(g)

            def pull_one(self):
                """Advance one unit from the first non-blocked generator
                (blocked heads are skipped, order otherwise preserved).
                Returns cost (us), 0.0 if all blocked, None if exhausted."""
                idx = 0
                pl = state.get("pulls", {}).get(state["cur_pair"])
                while idx < len(self.gens):
                    try:
                        cost = next(self.gens[idx])
                    except StopIteration:
                        self.gens.pop(idx)
                        continue
                    if cost is None:
                        idx += 1        # gated — try the next generator
                        continue
                    if pl is not None:
                        pl[0] += cost
                        pl[1] += 1
                    return cost
                if pl is not None:
                    pl[2] += 1
                return None if not self.gens else 0.0

            def drain(self):
                while self.pull_one() is not None:
                    pass

        feeder = Feeder()

        # ---------------- attention ----------------
        def attention(pr):
            state["pulls"] = state.get("pulls", {})
            state["pulls"][pr] = [0.0, 0, 0]   # us pulled, n pulled, n blocked
            state["cur_pair"] = pr
            qT, kT = qkT[pr]
            slab = pr // 4
            pending1 = []   # stage1: normalize (DVE)
            pending2 = []   # stage2: transpose + yT copy (PE+DVE)

            def emit_stage1(y_e, y_o, s, i):
                rc_e = pool_rc.tile([P, 1], F32, tag="rc", name="rc_e")
                rc_o = pool_rc.tile([P, 1], F32, tag="rc", name="rc_o")
                yn = pool_yn.tile([P, P], BF16, tag="yn", name="yn")
                with nc.allow_low_precision(reason="softmax normalize"):
                    nc.vector.reciprocal(rc_e[:], y_e[:, 65 * s + D:65 * s + D + 1])
                    nc.vector.reciprocal(rc_o[:], y_o[:, 65 * s + D:65 * s + D + 1])
                    nc.vector.tensor_scalar(
                        yn[:, 0:D], y_e[:, 65 * s:65 * s + D], rc_e[:, 0:1],
                        None, op0=mybir.AluOpType.mult)
                    nc.vector.tensor_scalar(
                        yn[:, D:2 * D], y_o[:, 65 * s:65 * s + D], rc_o[:, 0:1],
                        None, op0=mybir.AluOpType.mult)
                pending2.append((yn, i))

            def emit_stage2(yn, i):
                # PE transpose (DMA-engine transposes would head-block either
                # hwdge queue on the yn dependency: SP deadlocks vs wv/out
                # DMAs, ACT stalls exp dispatch)
                tr = psum_mm.tile([P, 1024], BF16, tag="mm", name="tr")
                nc.tensor.transpose(tr[:, 0:P], yn[:], identb_t[:])
                nc.vector.tensor_copy(yT_sb[pr][:, i * P:(i + 1) * P],
                                      tr[:, 0:P])
                state["tr"][pr] = i + 1
                if pr == PAIRS - 1:
                    state["tr7"] = i + 1

            debt = [0.0]

            def fill(extra=0.0):
                # in the last pair every pulled unit shortens the otherwise
                # ACT-idle projection tail: pull as hard as supply allows
                last = pr == PAIRS - 1
                debt[0] += extra + (0.25 if last else 0.0)
                pulls = 0
                while debt[0] > 0 and pulls < (4 if last else 2):
                    cost = feeder.pull_one()
                    if not cost:
                        break
                    debt[0] -= cost
                    pulls += 1
                debt[0] = min(debt[0], 8.0 if last else 4.0)

            for qb in range(NQB):
                q0 = qb * QB
                # this qb's q/k slices and v chunks must already be emitted
                spins = 0
                while (state["qk_prog"][pr] < qb or
                       state["v_prog"][slab] < min(4 * qb + 3, TKC - 1)):
                    c_ = feeder.pull_one()
                    spins += 1
                    if c_ is None or spins > 100000:
                        raise RuntimeError("feeder stuck before qk/v ready")
                y_e = psum_y.tile([P, 512], F32, tag="y", name="y_e")
                y_o = psum_y.tile([P, 512], F32, tag="y", name="y_o")

                def emit_pv(ex, n0, c, y_e=y_e, y_o=y_o, qb=qb):
                    s_min = max(0, c - 4 * qb)
                    for s in range(s_min, SUB):
                        # start=True clears has_written for the WHOLE bank, so
                        # only the tile's very first matmul may set it; later
                        # regions first-write via the cleared per-element bits
                        st_ = (c == 0 and s == 0)
                        sp_ = (c == 4 * qb + s)
                        nc.tensor.matmul(
                            y_e[:, 65 * s:65 * s + 65],
                            ex[:, s * P:(s + 1) * P],
                            v_t[c][:, 2 * pr, :],
                            start=st_, stop=sp_, skip_group_check=True)
                        nc.tensor.matmul(
                            y_o[:, 65 * s:65 * s + 65],
                            ex[:, QB + s * P - n0:QB + (s + 1) * P - n0],
                            v_t[c][:, 2 * pr + 1, :],
                            start=st_, stop=sp_, skip_group_check=True)
                        if sp_:
                            pending1.append((y_e, y_o, s, 4 * qb + s))

                pv_queue = []
                for c in range(4 * qb + 4):
                    n0 = max(0, c * P - q0)
                    sT = psum_s.tile([P, 2 * QB], F32, tag="sT", name="sT")
                    nc.tensor.matmul(
                        sT[:, n0:QB],
                        kT[0:D, c * P:(c + 1) * P],
                        qT[0:D, q0 + n0:q0 + QB],
                        start=True, stop=True, tile_position=(0, 0))
                    nc.tensor.matmul(
                        sT[:, QB:2 * QB - n0],
                        kT[D:2 * D, c * P:(c + 1) * P],
                        qT[D:2 * D, q0 + n0:q0 + QB],
                        start=True, stop=True, tile_position=(D, 0))
                    if pending2:
                        emit_stage2(*pending2.pop(0))
                    if pending1:
                        emit_stage1(*pending1.pop(0))
                    ex = pool_ex.tile([P, 2 * QB], BF16, tag="ex", name="ex")
                    nc.scalar.activation(ex[:, n0:2 * QB - n0],
                                         sT[:, n0:2 * QB - n0],
                                         mybir.ActivationFunctionType.Exp,
                                         scale=scale)
                    if c * P >= q0:   # diagonal 128-block: causal 0/1 mask
                        nc.vector.tensor_mul(ex[:, n0:n0 + P],
                                             ex[:, n0:n0 + P], mask_t[:])
                        nc.vector.tensor_mul(ex[:, QB:QB + P],
                                             ex[:, QB:QB + P], mask_t[:])
                    # debt-paced feeder fill: keep PE busy while ACT exps
                    s_min = max(0, c - 4 * qb)
                    act_c = (2 * QB - 2 * n0) * _ACT_US + 0.30
                    pe_c = (2 * (QB - n0) +
                            2 * (SUB - s_min) * 65) * _MM_US + 0.06
                    fill(act_c - pe_c)
                    # software pipeline (depth 2): PV lags the S/exp front by
                    # two chunks so the PE never couples to exp completion
                    pv_queue.append((ex, n0, c))
                    if len(pv_queue) > 1:
                        emit_pv(*pv_queue.pop(0))
                for it in pv_queue:
                    emit_pv(*it)
                # the y_e/y_o PSUM slots are recycled by the next qb's
                # allocation: every pending normalize reading them must be
                # emitted before that (stage2 may stay pending)
                while pending1:
                    emit_stage1(*pending1.pop(0))
                    feeder.pull_one()
            # drain transposes, interleaving feeder units
            while pending2:
                emit_stage2(*pending2.pop(0))
                feeder.pull_one()

        # ---------------- program ----------------
        # lead-in: pair 0's first q/k slice + first 4 v chunks directly
        g0 = gen_qkv(0)
        while state["qk_prog"].get(0, -1) < 0:
            next(g0)
        gv0 = gen_v(0)
        while state["v_prog"][0] < SUB - 1:
            next(gv0)
        feeder.push(g0)
        feeder.push(gv0)
        feeder.push(gen_qkv(1))
        feeder.push(gen_qkv(2))
        feeder.push(gen_qkv(3))
        feeder.push(gen_v(1))
        feeder.push(gen_qkv(4))
        feeder.push(gen_qkv(5))
        feeder.push(gen_qkv(6))
        feeder.push(gen_qkv(7))
        feeder.push(gen_proj_half(0))
        feeder.push(gen_proj_half(1))

        for pr in range(PAIRS):
            # barrier: this pair's first q/k slice must be emitted
            spins = 0
            while state["qk_prog"].get(pr, -1) < 0:
                c_ = feeder.pull_one()
                spins += 1
                if c_ is None or spins > 100000:
                    raise RuntimeError("feeder stuck before qk ready")
            attention(pr)
        feeder.drain()
        import os
        if os.environ.get("FEED_DEBUG"):
            for k in sorted(state.get("pulls", {})):
                us, n, blocked = state["pulls"][k]
                print(f"pair {k}: pulled {us:.1f}us in {n} units, "
                      f"{blocked} dry pulls")

    nc.compile()
    return nc


def make_const_inputs():
    ident = np.eye(P, dtype=np.float32)
    # S^T diagonal block mask: valid iff tq_local >= tk_local
    mask = np.triu(np.ones((P, P), np.float32)).astype(BF)
    return ident, mask


def make_in_maps(inputs, n_cores=8):
    """Host-side marshalling: shard x over batch, convert to bf16, transpose
    x, pack w_attn's q|k chunks into contiguous lhsT tiles."""
    x = np.asarray(inputs["x"], dtype=np.float32)
    w_attn = np.asarray(inputs["w_attn"], dtype=np.float32)
    w_proj = np.asarray(inputs["w_proj"], dtype=np.float32)
    b_attn = np.asarray(inputs.get("b_attn", 0), dtype=np.float32)
    b_proj = np.asarray(inputs.get("b_proj", 0), dtype=np.float32)
    B, T, C = x.shape

    wqk = w_attn[:, :2 * C]        # [C, 2C]
    # chunk m tile [p, j*128+n] = w_attn[j*128+p, m*128+n]
    wqk_packed = np.ascontiguousarray(
        wqk.reshape(C // P, P, 2 * C // P, P)     # [j, p, m, n]
        .transpose(2, 1, 0, 3)                    # [m, p, j, n]
        .reshape(2 * C, C)).astype(BF)
    wv = np.ascontiguousarray(w_attn[:, 2 * C:]).astype(BF)
    wp = np.ascontiguousarray(w_proj).astype(BF)
    ident, mask = make_const_inputs()

    wba = bool(np.any(b_attn != 0))
    wbp = bool(np.any(b_proj != 0))
    in_maps = []
    for i in range(n_cores):
        m = {"xT": np.ascontiguousarray(x[i].T).astype(BF),
             "wqk": wqk_packed, "wv": wv, "wp": wp,
             "ident": ident, "identb": ident.astype(BF), "mask": mask}
        if wba:
            m["b_attn"] = b_attn.reshape(1, -1).astype(BF)
        if wbp:
            m["b_proj"] = b_proj.reshape(1, -1).astype(BF)
        in_maps.append(m)
    return in_maps


_CACHE = {}


def _get_program(T, C, H, wba, wbp, n_cores):
    key = (T, C, H, wba, wbp, n_cores)
    if key not in _CACHE:
        _CACHE[key] = build_program(T=T, C=C, H=H, n_cores=n_cores,
                                    with_bias_attn=wba, with_bias_proj=wbp)
    return _CACHE[key]


def kernel(x, w_attn, b_attn, w_proj, b_proj):
    x = np.asarray(x, dtype=np.float32)
    B, T, C = x.shape
    H = 16
    n_cores = 8
    assert B == n_cores

    inputs = {"x": x, "w_attn": w_attn, "b_attn": b_attn,
              "w_proj": w_proj, "b_proj": b_proj}
    in_maps = make_in_maps(inputs, n_cores)
    wba = "b_attn" in in_maps[0]
    wbp = "b_proj" in in_maps[0]
    nc = _get_program(T, C, H, wba, wbp, n_cores)

    res = run_bass_kernel_spmd(nc, in_maps, list(range(n_cores)))
    return np.stack([res.results[i]["out"] for i in range(n_cores)], axis=0)
